# revision 27
# baseline (speedup 1.0000x reference)
"""LSTM (single layer, final hidden state) on 8 Trainium2 NeuronCores.

Reference computation (per batch row b):
    gx[t] = x[t] @ w_ih.T + (b_ih + b_hh)
    g     = gx[t] + h @ w_hh.T          # [B, 4H], gate order i,f,g,o
    i,f,o = sigmoid(...), g_c = tanh(...)
    c     = f*c + i*g_c
    h     = o * tanh(c)
returns h after T steps, shape [1, B, H].

Sharding: data-parallel over batch B=256 -> 8 cores x 32. Weights replicated.

Per-core layout ("packed"): partition p = 32*j + b, where j in [0,4) indexes
an H-quarter (H index = 64*j + s, s in [0,64)) and b in [0,32) is the local
batch.  All elementwise tiles are [128, *]:
    c, h            [128, 64]   c[32j+b, s] = C[b, 64j+s]
    gate psum       [128, 256]  cols 64*q+s with q order (i, f, o, g)
Gates are produced by 4 column-tiled concurrent matmuls (tile_position
(0,32j)), accumulating 4 K-rounds: bias (K=1 ones trick), x_t (K=128),
h chunk0 (K=128), h chunk1 (K=128).  The stationary operands are the small
[K,32] transposes of x_t / h, so weight loads are cheap; the big W tiles
stream through the moving port of 4 column groups concurrently.

h -> h.T for the next step is done with 4 concurrent row+col-tiled PE
transposes ([32,64] blocks at tile_position (32j, 64*(j%2))) into one PSUM
tile, then one DVE copy to SBUF.
"""

import os
import sys

import numpy as np

B_TOT, T_FULL, I_DIM, H = 256, 1024, 128, 256
NCORES = 8
B = B_TOT // NCORES  # 32 per core
NJ = 4  # H quarters
S = H // NJ  # 64
# column order within a gate-quarter: (i, f, o, g_cell); row bases in w/b
Q_ROWBASE = (0, 256, 768, 512)


def _ensure_paths():
    for p in ("/opt/trn_rl_repo",):
        if os.path.isdir(p) and p not in sys.path:
            sys.path.append(p)


def _prep_weights(w_ih, w_hh, b_ih, b_hh):
    """Host-side permutation of weights into the packed rhs layouts."""
    wih_p = np.empty((I_DIM, NJ, 4 * S), np.float32)  # [128, 4, 256]
    whh_p = np.empty((128, 2, NJ, 4 * S), np.float32)  # [128, u, j, 256]
    bias_p = np.empty((1, NJ, 4 * S), np.float32)  # [1, 4, 256]
    bsum = (b_ih + b_hh).astype(np.float32)
    # DVE 32x32 block-transpose of packed h puts H-input index
    # 64*(k//32) + 32*u + (k%32) at partition k of lhsT column-group u.
    k = np.arange(128)
    hperm = [64 * (k // 32) + 32 * u + (k % 32) for u in range(2)]
    for q, rb in enumerate(Q_ROWBASE):
        for j in range(NJ):
            rows = slice(rb + S * j, rb + S * j + S)
            wih_p[:, j, S * q : S * q + S] = w_ih[rows, :].T
            for u in range(2):
                whh_p[:, u, j, S * q : S * q + S] = w_hh[rows, :][:, hperm[u]].T
            bias_p[0, j, S * q : S * q + S] = bsum[rows]
    ident = np.zeros((128, 32), np.float32)
    for p in range(128):
        ident[p, p % 32] = 1.0
    return wih_p, whh_p, bias_p, ident


def build_nc(T=T_FULL, TC=32, debug=False):
    """Build the per-core Bass program (SPMD: same program on all cores)."""
    _ensure_paths()
    import concourse.bacc as bacc
    import concourse.mybir as mybir
    import concourse.tile as tile
    from contextlib import ExitStack

    fp32 = mybir.dt.float32
    bf16 = mybir.dt.bfloat16
    AF = mybir.ActivationFunctionType

    assert T % TC == 0 and TC % 8 == 0

    nc = bacc.Bacc("TRN2", target_bir_lowering=False, debug=debug)

    x_d = nc.dram_tensor("x", [B, T, I_DIM], fp32, kind="ExternalInput").ap()
    h0_d = nc.dram_tensor("h0", [B, H], bf16, kind="ExternalInput").ap()
    c0_d = nc.dram_tensor("c0", [B, H], fp32, kind="ExternalInput").ap()
    # x/h weights in bf16: matmuls stream at 1 cycle/row at any N and keep the
    # 4-way PE column-group concurrency (fp32 is 2 half-speed passes; fp32r
    # forbids dst partitions != 0, which the column groups need).  The bias
    # round stays fp32/exact.
    wih_d = nc.dram_tensor(
        "wih_p", [I_DIM, NJ, 4 * S], bf16, kind="ExternalInput"
    ).ap()
    whh_d = nc.dram_tensor(
        "whh_p", [128, 2, NJ, 4 * S], bf16, kind="ExternalInput"
    ).ap()
    bias_d = nc.dram_tensor(
        "bias_p", [1, NJ, 4 * S], fp32, kind="ExternalInput"
    ).ap()
    ident_d = nc.dram_tensor("ident", [128, 32], fp32, kind="ExternalInput").ap()
    hn_d = nc.dram_tensor("hn", [B, H], fp32, kind="ExternalOutput").ap()

    with tile.TileContext(nc) as tc, ExitStack() as ctx:
        consts = ctx.enter_context(tc.tile_pool(name="consts", bufs=1))
        states = ctx.enter_context(tc.tile_pool(name="states", bufs=1))
        lhsT_pool = ctx.enter_context(tc.tile_pool(name="lhsT", bufs=3))
        x_pool = ctx.enter_context(tc.tile_pool(name="xstream", bufs=2))
        xT_pool = ctx.enter_context(tc.tile_pool(name="xT", bufs=3))
        ew_pool = ctx.enter_context(tc.tile_pool(name="ew", bufs=3))
        g_psum = ctx.enter_context(tc.tile_pool(name="g_psum", bufs=2, space="PSUM"))
        xt_psum = ctx.enter_context(tc.tile_pool(name="xt_psum", bufs=2, space="PSUM"))

        # ---- constants ----
        wih_sb = consts.tile([I_DIM, NJ, 4 * S], bf16, name="wih_sb")
        nc.sync.dma_start(out=wih_sb, in_=wih_d)
        whh_sb = consts.tile([128, 2, NJ, 4 * S], bf16, name="whh_sb")
        nc.sync.dma_start(out=whh_sb, in_=whh_d)
        bias_sb = consts.tile([1, NJ, 4 * S], fp32, name="bias_sb")
        nc.sync.dma_start(out=bias_sb, in_=bias_d)
        ident_sb = consts.tile([128, 32], fp32, name="ident_sb")
        nc.sync.dma_start(out=ident_sb, in_=ident_d)
        ones_sb = consts.tile([1, 32], fp32, name="ones_sb")
        nc.vector.memset(ones_sb, 1.0)
        # rhs of the zero-contribution "keep the PE p-state warm" matmuls.
        zeros_sb = consts.tile([1, 4 * S], fp32, name="zeros_sb")
        nc.vector.memset(zeros_sb, 0.0)

        # ---- state init (packed) ----
        c_sb = states.tile([128, S], fp32, name="c_sb")
        # h only feeds the gate matmuls (via the transpose), so it lives in
        # bf16; the final step writes a separate fp32 copy for the output.
        h_sb = states.tile([128, S], bf16, name="h_sb")
        for j in range(NJ):
            nc.sync.dma_start(
                out=c_sb[32 * j : 32 * j + 32, :], in_=c0_d[:, S * j : S * j + S]
            )
            nc.sync.dma_start(
                out=h_sb[32 * j : 32 * j + 32, :], in_=h0_d[:, S * j : S * j + S]
            )

        def emit_hT():
            """DVE 32x32 block transpose of packed h -> lhsT column groups.

            hv[32J+y, 32u+x] = h[32J+x, 32u+y] = H[x, 64J+32u+y]; so
            hv[:, 32u:32u+32] is a [K=128, M=32] stationary operand whose
            K-rows enumerate H-inputs in the order 64*(k//32)+32u+(k%32) —
            whh_p is host-permuted to match.
            """
            hT = lhsT_pool.tile([128, 2 * 32], bf16, name="hT")
            nc.vector.transpose(out=hT, in_=h_sb)
            return hT

        hT = emit_hT()

        n_chunks = T // TC

        def fetch(ch):
            """Start the async HBM read of one x chunk (prefetched 1 ahead)."""
            x_sb = x_pool.tile([B, TC, I_DIM], fp32, name="x_sb")
            nc.sync.dma_start(out=x_sb, in_=x_d[:, ch * TC : (ch + 1) * TC, :])
            return x_sb

        def prep_chunk(x_sb):
            """PE-transpose a chunk's x into per-step lhsT tiles."""
            xT_tiles = []
            for g8 in range(TC // 8):
                xt_ps = xt_psum.tile([128, 8 * 32], fp32, name="xt_ps")
                for v in range(8):
                    nc.tensor.transpose(
                        out=xt_ps[:, 32 * v : 32 * v + 32],
                        in_=x_sb[:, g8 * 8 + v, :],
                        identity=ident_sb[0:32, :],
                        tile_position=(0, 0),
                    )
                xT_sb = xT_pool.tile([128, 8 * 32], bf16, name="xT_sb")
                nc.vector.tensor_copy(out=xT_sb, in_=xt_ps)
                xT_tiles.append(xT_sb)
            return xT_tiles

        def start_rounds(xT_sl):
            """Open a step's psum accumulation: bias + x rounds (h-independent,
            so they run on the PE as soon as the bank frees, well before hT)."""
            g_ps = g_psum.tile([128, 4 * S], fp32, name="g_ps")
            for j in range(NJ):
                nc.tensor.matmul(
                    g_ps[32 * j : 32 * j + 32, :], ones_sb, bias_sb[0:1, j, :],
                    start=True, stop=False,
                    tile_position=(0, 32 * j), skip_group_check=True,
                )
            for j in range(NJ):
                nc.tensor.matmul(
                    g_ps[32 * j : 32 * j + 32, :], xT_sl, wih_sb[:, j, :],
                    start=False, stop=False,
                    tile_position=(0, 32 * j), skip_group_check=True,
                )
            return g_ps

        x_next = fetch(0)
        g_ps = None
        for ch in range(n_chunks):
            x_cur = x_next
            if ch + 1 < n_chunks:
                x_next = fetch(ch + 1)
            xT_tiles = prep_chunk(x_cur)
            if g_ps is None:
                g_ps = start_rounds(xT_tiles[0][:, 0:32])
            for u in range(TC):
                t = ch * TC + u
                # h rounds: the only h_{t-1}-dependent matmuls; round-major
                # across the 4 PE column groups for concurrency.
                for rnd in range(2):
                    for j in range(NJ):
                        nc.tensor.matmul(
                            g_ps[32 * j : 32 * j + 32, :],
                            hT[:, 32 * rnd : 32 * rnd + 32],
                            whh_sb[:, rnd, j, :],
                            start=False, stop=(rnd == 1),
                            tile_position=(0, 32 * j), skip_group_check=True,
                        )
                # gates: cols [0:64]=i [64:128]=f [128:192]=o [192:256]=g_cell
                sig = ew_pool.tile([128, 3 * S], fp32, name="sig")
                # i,f first: the c-path only needs these + tanh(g); o is
                # consumed much later (h = o*tanh(c)), so its sigmoid runs
                # off the spine while DVE does the c update.
                nc.scalar.activation(
                    sig[:, 0 : 2 * S], g_ps[:, 0 : 2 * S], AF.Sigmoid
                )
                tg = ew_pool.tile([128, S], fp32, name="tg")
                nc.scalar.activation(tg, g_ps[:, 3 * S : 4 * S], AF.Tanh)
                nc.vector.tensor_mul(c_sb, sig[:, S : 2 * S], c_sb)
                t1 = ew_pool.tile([128, S], fp32, name="t1")
                nc.vector.tensor_mul(t1, sig[:, 0:S], tg)
                nc.scalar.activation(
                    sig[:, 2 * S : 3 * S], g_ps[:, 2 * S : 3 * S], AF.Sigmoid
                )
                nc.vector.tensor_add(c_sb, c_sb, t1)
                tcc = ew_pool.tile([128, S], fp32, name="tcc")
                nc.scalar.activation(tcc, c_sb, AF.Tanh)
                nc.vector.tensor_mul(h_sb, sig[:, 2 * S : 3 * S], tcc)
                if t < T - 1:
                    hT = emit_hT()
                else:
                    # full-precision copy of the final h for the output
                    hf_sb = states.tile([128, S], fp32, name="hf_sb")
                    nc.vector.tensor_mul(hf_sb, sig[:, 2 * S : 3 * S], tcc)
                if u < TC - 1:
                    v = u + 1
                    g_next = start_rounds(
                        xT_tiles[v // 8][:, 32 * (v % 8) : 32 * (v % 8) + 32]
                    )
                    # Zero-contribution matmuls (out += src_row x zeros = 0)
                    # chained on the elementwise intermediates: the PE touches
                    # work every ~400ns through the elementwise window, so the
                    # h rounds never pay the cold p-state penalty (measured
                    # 635ns vs ~110ns warm).  Shape mirrors the bias round
                    # (K=1, M=32, N=256), which codegen accepts.
                    for src in (tg, t1, c_sb, tcc):
                        nc.tensor.matmul(
                            g_next[0:32, :], src[0:1, 0:32], zeros_sb,
                            start=False, stop=False,
                            tile_position=(0, 0), skip_group_check=True,
                        )
                    g_ps = g_next
                else:
                    g_ps = None  # reopened at the next chunk top

        # ---- write back final h (unpack) ----
        for j in range(NJ):
            nc.sync.dma_start(
                out=hn_d[:, S * j : S * j + S], in_=hf_sb[32 * j : 32 * j + 32, :]
            )

    nc.compile()
    return nc


def _shard_inputs(x, h0, c0, w_ih, w_hh, b_ih, b_hh, T=T_FULL):
    import ml_dtypes

    bf16 = ml_dtypes.bfloat16
    wih_p, whh_p, bias_p, ident = _prep_weights(
        np.asarray(w_ih, np.float32),
        np.asarray(w_hh, np.float32),
        np.asarray(b_ih, np.float32),
        np.asarray(b_hh, np.float32),
    )
    wih_p = wih_p.astype(bf16)
    whh_p = whh_p.astype(bf16)
    x = np.asarray(x, np.float32)
    h0 = np.asarray(h0, np.float32)
    c0 = np.asarray(c0, np.float32)
    in_maps = []
    for k in range(NCORES):
        bs = slice(B * k, B * (k + 1))
        in_maps.append(
            {
                "x": np.ascontiguousarray(x[bs, :T, :]),
                "h0": np.ascontiguousarray(h0[0, bs, :]).astype(bf16),
                "c0": np.ascontiguousarray(c0[0, bs, :]),
                "wih_p": wih_p,
                "whh_p": whh_p,
                "bias_p": bias_p,
                "ident": ident,
            }
        )
    return in_maps


_NC_CACHE = {}


def run_hw(x, h0, c0, w_ih, w_hh, b_ih, b_hh, T=T_FULL, TC=32, trace=False):
    _ensure_paths()
    from concourse.bass_utils import run_bass_kernel_spmd

    key = (T, TC)
    if key not in _NC_CACHE:
        _NC_CACHE[key] = build_nc(T=T, TC=TC)
    nc = _NC_CACHE[key]
    in_maps = _shard_inputs(x, h0, c0, w_ih, w_hh, b_ih, b_hh, T=T)
    res = run_bass_kernel_spmd(nc, in_maps, list(range(NCORES)), trace=trace)
    hn = np.stack([res.results[k]["hn"] for k in range(NCORES)], axis=0)
    return hn.reshape(1, B_TOT, H), res


def kernel(x, h0, c0, w_ih, w_hh, b_ih, b_hh):
    out, _ = run_hw(x, h0, c0, w_ih, w_hh, b_ih, b_hh)
    return out.astype(np.float32)


def _np_reference(x, h0, c0, w_ih, w_hh, b_ih, b_hh, T=None):
    """Numpy oracle for development (matches reference.py)."""
    x = np.asarray(x, np.float64)
    if T is not None:
        x = x[:, :T, :]
    h = np.asarray(h0, np.float64)[0]
    c = np.asarray(c0, np.float64)[0]
    gx = np.einsum("bti,gi->tbg", x, np.asarray(w_ih, np.float64)) + (
        np.asarray(b_ih, np.float64) + np.asarray(b_hh, np.float64)
    )
    W = np.asarray(w_hh, np.float64)

    def sg(v):
        return 1.0 / (1.0 + np.exp(-v))

    for t in range(x.shape[1]):
        g = gx[t] + h @ W.T
        i = sg(g[:, 0:256])
        f = sg(g[:, 256:512])
        gg = np.tanh(g[:, 512:768])
        o = sg(g[:, 768:1024])
        c = f * c + i * gg
        h = o * np.tanh(c)
    return h[None].astype(np.float32)



# revision 32
# speedup vs baseline: 1.4657x; 1.4657x over previous
"""LSTM (single layer, final hidden state) on 8 Trainium2 NeuronCores.

Reference computation (per batch row b):
    gx[t] = x[t] @ w_ih.T + (b_ih + b_hh)
    g     = gx[t] + h @ w_hh.T          # [B, 4H], gate order i,f,g,o
    i,f,o = sigmoid(...), g_c = tanh(...)
    c     = f*c + i*g_c
    h     = o * tanh(c)
returns h after T steps, shape [1, B, H].

Sharding: data-parallel over batch B=256 -> 8 cores x 32. Weights replicated.

Per-core layout ("packed"): partition p = 32*j + b, where j in [0,4) indexes
an H-quarter (H index = 64*j + s, s in [0,64)) and b in [0,32) is the local
batch.  All elementwise tiles are [128, *]:
    c, h            [128, 64]   c[32j+b, s] = C[b, 64j+s]
    gate psum       [128, 256]  cols 64*q+s with q order (i, f, o, g)
Gates are produced by 4 column-tiled concurrent matmuls (tile_position
(0,32j)), accumulating 4 K-rounds: bias (K=1 ones trick), x_t (K=128),
h chunk0 (K=128), h chunk1 (K=128).  The stationary operands are the small
[K,32] transposes of x_t / h, so weight loads are cheap; the big W tiles
stream through the moving port of 4 column groups concurrently.

h -> h.T for the next step is done with 4 concurrent row+col-tiled PE
transposes ([32,64] blocks at tile_position (32j, 64*(j%2))) into one PSUM
tile, then one DVE copy to SBUF.
"""

import os
import sys

import numpy as np

B_TOT, T_FULL, I_DIM, H = 256, 1024, 128, 256
NCORES = 8
B = B_TOT // NCORES  # 32 per core
NJ = 4  # H quarters
S = H // NJ  # 64
# column order within a gate-quarter: (i, f, o, g_cell); row bases in w/b
Q_ROWBASE = (0, 256, 768, 512)


def _ensure_paths():
    for p in ("/opt/trn_rl_repo",):
        if os.path.isdir(p) and p not in sys.path:
            sys.path.append(p)


def _prep_weights(w_ih, w_hh, b_ih, b_hh):
    """Host-side permutation of weights into the packed rhs layouts."""
    wih_p = np.empty((I_DIM, NJ, 4 * S), np.float32)  # [128, 4, 256]
    whh_p = np.empty((128, 2, NJ, 4 * S), np.float32)  # [128, u, j, 256]
    bias_p = np.empty((1, NJ, 4 * S), np.float32)  # [1, 4, 256]
    bsum = (b_ih + b_hh).astype(np.float32)
    # DVE 32x32 block-transpose of packed h puts H-input index
    # 64*(k//32) + 32*u + (k%32) at partition k of lhsT column-group u.
    k = np.arange(128)
    hperm = [64 * (k // 32) + 32 * u + (k % 32) for u in range(2)]
    for q, rb in enumerate(Q_ROWBASE):
        for j in range(NJ):
            rows = slice(rb + S * j, rb + S * j + S)
            wih_p[:, j, S * q : S * q + S] = w_ih[rows, :].T
            for u in range(2):
                whh_p[:, u, j, S * q : S * q + S] = w_hh[rows, :][:, hperm[u]].T
            bias_p[0, j, S * q : S * q + S] = bsum[rows]
    ident = np.zeros((128, 32), np.float32)
    for p in range(128):
        ident[p, p % 32] = 1.0
    return wih_p, whh_p, bias_p, ident


def build_nc(T=T_FULL, TC=32, debug=False):
    """Build the per-core Bass program (SPMD: same program on all cores)."""
    _ensure_paths()
    import concourse.bacc as bacc
    import concourse.mybir as mybir
    import concourse.tile as tile
    from contextlib import ExitStack

    fp32 = mybir.dt.float32
    bf16 = mybir.dt.bfloat16
    AF = mybir.ActivationFunctionType

    assert T % TC == 0 and TC % 8 == 0

    nc = bacc.Bacc("TRN2", target_bir_lowering=False, debug=debug)

    x_d = nc.dram_tensor("x", [B, T, I_DIM], fp32, kind="ExternalInput").ap()
    h0_d = nc.dram_tensor("h0", [B, H], bf16, kind="ExternalInput").ap()
    c0_d = nc.dram_tensor("c0", [B, H], fp32, kind="ExternalInput").ap()
    # x/h weights in bf16: matmuls stream at 1 cycle/row at any N and keep the
    # 4-way PE column-group concurrency (fp32 is 2 half-speed passes; fp32r
    # forbids dst partitions != 0, which the column groups need).  The bias
    # round stays fp32/exact.
    wih_d = nc.dram_tensor(
        "wih_p", [I_DIM, NJ, 4 * S], bf16, kind="ExternalInput"
    ).ap()
    whh_d = nc.dram_tensor(
        "whh_p", [128, 2, NJ, 4 * S], bf16, kind="ExternalInput"
    ).ap()
    bias_d = nc.dram_tensor(
        "bias_p", [1, NJ, 4 * S], fp32, kind="ExternalInput"
    ).ap()
    ident_d = nc.dram_tensor("ident", [128, 32], fp32, kind="ExternalInput").ap()
    hn_d = nc.dram_tensor("hn", [B, H], fp32, kind="ExternalOutput").ap()

    with tile.TileContext(nc) as tc, ExitStack() as ctx:
        consts = ctx.enter_context(tc.tile_pool(name="consts", bufs=1))
        states = ctx.enter_context(tc.tile_pool(name="states", bufs=1))
        lhsT_pool = ctx.enter_context(tc.tile_pool(name="lhsT", bufs=3))
        x_pool = ctx.enter_context(tc.tile_pool(name="xstream", bufs=2))
        xT_pool = ctx.enter_context(tc.tile_pool(name="xT", bufs=3))
        ew_pool = ctx.enter_context(tc.tile_pool(name="ew", bufs=3))
        g_psum = ctx.enter_context(tc.tile_pool(name="g_psum", bufs=2, space="PSUM"))
        xt_psum = ctx.enter_context(tc.tile_pool(name="xt_psum", bufs=2, space="PSUM"))

        # ---- constants ----
        wih_sb = consts.tile([I_DIM, NJ, 4 * S], bf16, name="wih_sb")
        nc.sync.dma_start(out=wih_sb, in_=wih_d)
        whh_sb = consts.tile([128, 2, NJ, 4 * S], bf16, name="whh_sb")
        nc.sync.dma_start(out=whh_sb, in_=whh_d)
        bias_sb = consts.tile([1, NJ, 4 * S], fp32, name="bias_sb")
        nc.sync.dma_start(out=bias_sb, in_=bias_d)
        ident_sb = consts.tile([128, 32], fp32, name="ident_sb")
        nc.sync.dma_start(out=ident_sb, in_=ident_d)
        ones_sb = consts.tile([1, 32], fp32, name="ones_sb")
        nc.vector.memset(ones_sb, 1.0)
        # rhs of the zero-contribution "keep the PE p-state warm" matmuls.
        # bf16 so each dummy is one single-cycle-per-row pass (fp32 dummies
        # measured 427ns x 2 passes each -- worse than the cold clock).
        zeros_sb = consts.tile([1, 4 * S], bf16, name="zeros_sb")
        nc.vector.memset(zeros_sb, 0.0)

        # ---- state init (packed) ----
        c_sb = states.tile([128, S], fp32, name="c_sb")
        # h only feeds the gate matmuls (via the transpose), so it lives in
        # bf16; the final step writes a separate fp32 copy for the output.
        h_sb = states.tile([128, S], bf16, name="h_sb")
        for j in range(NJ):
            nc.sync.dma_start(
                out=c_sb[32 * j : 32 * j + 32, :], in_=c0_d[:, S * j : S * j + S]
            )
            nc.sync.dma_start(
                out=h_sb[32 * j : 32 * j + 32, :], in_=h0_d[:, S * j : S * j + S]
            )

        def emit_hT():
            """DVE 32x32 block transpose of packed h -> lhsT column groups.

            hv[32J+y, 32u+x] = h[32J+x, 32u+y] = H[x, 64J+32u+y]; so
            hv[:, 32u:32u+32] is a [K=128, M=32] stationary operand whose
            K-rows enumerate H-inputs in the order 64*(k//32)+32u+(k%32) —
            whh_p is host-permuted to match.
            """
            hT = lhsT_pool.tile([128, 2 * 32], bf16, name="hT")
            nc.vector.transpose(out=hT, in_=h_sb)
            return hT

        hT = emit_hT()

        n_chunks = T // TC

        def fetch(ch):
            """Start the async HBM read of one x chunk (prefetched 1 ahead)."""
            x_sb = x_pool.tile([B, TC, I_DIM], fp32, name="x_sb")
            nc.sync.dma_start(out=x_sb, in_=x_d[:, ch * TC : (ch + 1) * TC, :])
            return x_sb

        def prep_chunk(x_sb):
            """PE-transpose a chunk's x into per-step lhsT tiles."""
            xT_tiles = []
            for g8 in range(TC // 8):
                xt_ps = xt_psum.tile([128, 8 * 32], fp32, name="xt_ps")
                for v in range(8):
                    nc.tensor.transpose(
                        out=xt_ps[:, 32 * v : 32 * v + 32],
                        in_=x_sb[:, g8 * 8 + v, :],
                        identity=ident_sb[0:32, :],
                        tile_position=(0, 0),
                    )
                xT_sb = xT_pool.tile([128, 8 * 32], bf16, name="xT_sb")
                nc.vector.tensor_copy(out=xT_sb, in_=xt_ps)
                xT_tiles.append(xT_sb)
            return xT_tiles

        def start_rounds(xT_sl):
            """Open a step's psum accumulation: bias + x rounds (h-independent,
            so they run on the PE as soon as the bank frees, well before hT)."""
            g_ps = g_psum.tile([128, 4 * S], fp32, name="g_ps")
            for j in range(NJ):
                nc.tensor.matmul(
                    g_ps[32 * j : 32 * j + 32, :], ones_sb, bias_sb[0:1, j, :],
                    start=True, stop=False,
                    tile_position=(0, 32 * j), skip_group_check=True,
                )
            for j in range(NJ):
                nc.tensor.matmul(
                    g_ps[32 * j : 32 * j + 32, :], xT_sl, wih_sb[:, j, :],
                    start=False, stop=False,
                    tile_position=(0, 32 * j), skip_group_check=True,
                )
            return g_ps

        x_next = fetch(0)
        g_ps = None
        for ch in range(n_chunks):
            x_cur = x_next
            if ch + 1 < n_chunks:
                x_next = fetch(ch + 1)
            xT_tiles = prep_chunk(x_cur)
            if g_ps is None:
                g_ps = start_rounds(xT_tiles[0][:, 0:32])
            for u in range(TC):
                t = ch * TC + u
                # h rounds: the only h_{t-1}-dependent matmuls; round-major
                # across the 4 PE column groups for concurrency.  Column-split
                # i,f first so the sigmoid can start while the o,g half still
                # streams.
                for half in range(2):
                    cols = slice(2 * S * half, 2 * S * half + 2 * S)
                    for rnd in range(2):
                        for j in range(NJ):
                            nc.tensor.matmul(
                                g_ps[32 * j : 32 * j + 32, cols],
                                hT[:, 32 * rnd : 32 * rnd + 32],
                                whh_sb[:, rnd, j, cols],
                                start=False, stop=(rnd == 1),
                                tile_position=(0, 32 * j), skip_group_check=True,
                            )
                # gates: cols [0:64]=i [64:128]=f [128:192]=o [192:256]=g_cell
                sig = ew_pool.tile([128, 3 * S], fp32, name="sig")
                # i,f first: the c-path only needs these + tanh(g); o is
                # consumed much later (h = o*tanh(c)), so its sigmoid runs
                # off the spine while DVE does the c update.
                nc.scalar.activation(
                    sig[:, 0 : 2 * S], g_ps[:, 0 : 2 * S], AF.Sigmoid
                )
                tg = ew_pool.tile([128, S], fp32, name="tg")
                nc.scalar.activation(tg, g_ps[:, 3 * S : 4 * S], AF.Tanh)
                nc.vector.tensor_mul(c_sb, sig[:, S : 2 * S], c_sb)
                t1 = ew_pool.tile([128, S], fp32, name="t1")
                nc.vector.tensor_mul(t1, sig[:, 0:S], tg)
                nc.scalar.activation(
                    sig[:, 2 * S : 3 * S], g_ps[:, 2 * S : 3 * S], AF.Sigmoid
                )
                nc.vector.tensor_add(c_sb, c_sb, t1)
                # bf16: h is bf16 anyway, and bf16 lets tcc serve as the lhsT
                # of the warm-up dummy below.
                tcc = ew_pool.tile([128, S], bf16, name="tcc")
                nc.scalar.activation(tcc, c_sb, AF.Tanh)
                nc.vector.tensor_mul(h_sb, sig[:, 2 * S : 3 * S], tcc)
                if t < T - 1:
                    hT = emit_hT()
                else:
                    # full-precision copy of the final h for the output
                    hf_sb = states.tile([128, S], fp32, name="hf_sb")
                    nc.vector.tensor_mul(hf_sb, sig[:, 2 * S : 3 * S], tcc)
                if u < TC - 1:
                    v = u + 1
                    g_next = start_rounds(
                        xT_tiles[v // 8][:, 32 * (v % 8) : 32 * (v % 8) + 32]
                    )
                    # Zero-contribution bf16 matmuls (out += src_row x 0 = 0)
                    # chained on the tail elementwise tiles: they fire just
                    # before the h rounds, pulling the PE out of its cold
                    # p-state (cold first round measured ~630ns vs ~110 warm).
                    for src in (tcc, h_sb):
                        nc.tensor.matmul(
                            g_next[0:32, 0:S], src[0:1, 0:32], zeros_sb[0:1, 0:S],
                            start=False, stop=False,
                            tile_position=(0, 0), skip_group_check=True,
                        )
                    g_ps = g_next
                else:
                    g_ps = None  # reopened at the next chunk top

        # ---- write back final h (unpack) ----
        for j in range(NJ):
            nc.sync.dma_start(
                out=hn_d[:, S * j : S * j + S], in_=hf_sb[32 * j : 32 * j + 32, :]
            )

    nc.compile()
    return nc


def _shard_inputs(x, h0, c0, w_ih, w_hh, b_ih, b_hh, T=T_FULL):
    import ml_dtypes

    bf16 = ml_dtypes.bfloat16
    wih_p, whh_p, bias_p, ident = _prep_weights(
        np.asarray(w_ih, np.float32),
        np.asarray(w_hh, np.float32),
        np.asarray(b_ih, np.float32),
        np.asarray(b_hh, np.float32),
    )
    wih_p = wih_p.astype(bf16)
    whh_p = whh_p.astype(bf16)
    x = np.asarray(x, np.float32)
    h0 = np.asarray(h0, np.float32)
    c0 = np.asarray(c0, np.float32)
    in_maps = []
    for k in range(NCORES):
        bs = slice(B * k, B * (k + 1))
        in_maps.append(
            {
                "x": np.ascontiguousarray(x[bs, :T, :]),
                "h0": np.ascontiguousarray(h0[0, bs, :]).astype(bf16),
                "c0": np.ascontiguousarray(c0[0, bs, :]),
                "wih_p": wih_p,
                "whh_p": whh_p,
                "bias_p": bias_p,
                "ident": ident,
            }
        )
    return in_maps


_NC_CACHE = {}


def run_hw(x, h0, c0, w_ih, w_hh, b_ih, b_hh, T=T_FULL, TC=32, trace=False):
    _ensure_paths()
    from concourse.bass_utils import run_bass_kernel_spmd

    key = (T, TC)
    if key not in _NC_CACHE:
        _NC_CACHE[key] = build_nc(T=T, TC=TC)
    nc = _NC_CACHE[key]
    in_maps = _shard_inputs(x, h0, c0, w_ih, w_hh, b_ih, b_hh, T=T)
    res = run_bass_kernel_spmd(nc, in_maps, list(range(NCORES)), trace=trace)
    hn = np.stack([res.results[k]["hn"] for k in range(NCORES)], axis=0)
    return hn.reshape(1, B_TOT, H), res


def kernel(x, h0, c0, w_ih, w_hh, b_ih, b_hh):
    out, _ = run_hw(x, h0, c0, w_ih, w_hh, b_ih, b_hh)
    return out.astype(np.float32)


def _np_reference(x, h0, c0, w_ih, w_hh, b_ih, b_hh, T=None):
    """Numpy oracle for development (matches reference.py)."""
    x = np.asarray(x, np.float64)
    if T is not None:
        x = x[:, :T, :]
    h = np.asarray(h0, np.float64)[0]
    c = np.asarray(c0, np.float64)[0]
    gx = np.einsum("bti,gi->tbg", x, np.asarray(w_ih, np.float64)) + (
        np.asarray(b_ih, np.float64) + np.asarray(b_hh, np.float64)
    )
    W = np.asarray(w_hh, np.float64)

    def sg(v):
        return 1.0 / (1.0 + np.exp(-v))

    for t in range(x.shape[1]):
        g = gx[t] + h @ W.T
        i = sg(g[:, 0:256])
        f = sg(g[:, 256:512])
        gg = np.tanh(g[:, 512:768])
        o = sg(g[:, 768:1024])
        c = f * c + i * gg
        h = o * np.tanh(c)
    return h[None].astype(np.float32)



# revision 37
# speedup vs baseline: 1.7625x; 1.2025x over previous
"""LSTM (single layer, final hidden state) on 8 Trainium2 NeuronCores.

Reference computation (per batch row b):
    gx[t] = x[t] @ w_ih.T + (b_ih + b_hh)
    g     = gx[t] + h @ w_hh.T          # [B, 4H], gate order i,f,g,o
    i,f,o = sigmoid(...), g_c = tanh(...)
    c     = f*c + i*g_c
    h     = o * tanh(c)
returns h after T steps, shape [1, B, H].

Sharding: data-parallel over batch B=256 -> 8 cores x 32. Weights replicated.

Per-core layout ("packed"): partition p = 32*j + b, where j in [0,4) indexes
an H-quarter (H index = 64*j + s, s in [0,64)) and b in [0,32) is the local
batch.  All elementwise tiles are [128, *]:
    c, h            [128, 64]   c[32j+b, s] = C[b, 64j+s]
    gate psum       [128, 256]  cols 64*q+s with q order (i, f, o, g)
Gates are produced by 4 column-tiled concurrent matmuls (tile_position
(0,32j)), accumulating 4 K-rounds: bias (K=1 ones trick), x_t (K=128),
h chunk0 (K=128), h chunk1 (K=128).  The stationary operands are the small
[K,32] transposes of x_t / h, so weight loads are cheap; the big W tiles
stream through the moving port of 4 column groups concurrently.

h -> h.T for the next step is done with 4 concurrent row+col-tiled PE
transposes ([32,64] blocks at tile_position (32j, 64*(j%2))) into one PSUM
tile, then one DVE copy to SBUF.
"""

import os
import sys

import numpy as np

B_TOT, T_FULL, I_DIM, H = 256, 1024, 128, 256
NCORES = 8
B = B_TOT // NCORES  # 32 per core
NJ = 4  # H quarters
S = H // NJ  # 64
# column order within a gate-quarter: (i, f, o, g_cell); row bases in w/b
Q_ROWBASE = (0, 256, 768, 512)


def _ensure_paths():
    for p in ("/opt/trn_rl_repo",):
        if os.path.isdir(p) and p not in sys.path:
            sys.path.append(p)


def _prep_weights(w_ih, w_hh, b_ih, b_hh):
    """Host-side permutation of weights into the packed rhs layouts."""
    wih_p = np.empty((I_DIM, NJ, 4 * S), np.float32)  # [128, 4, 256]
    whh_p = np.empty((128, 2, NJ, 4 * S), np.float32)  # [128, u, j, 256]
    bias_p = np.empty((1, NJ, 4 * S), np.float32)  # [1, 4, 256]
    bsum = (b_ih + b_hh).astype(np.float32)
    # DVE 32x32 block-transpose of packed h puts H-input index
    # 64*(k//32) + 32*u + (k%32) at partition k of lhsT column-group u.
    k = np.arange(128)
    hperm = [64 * (k // 32) + 32 * u + (k % 32) for u in range(2)]
    for q, rb in enumerate(Q_ROWBASE):
        for j in range(NJ):
            rows = slice(rb + S * j, rb + S * j + S)
            wih_p[:, j, S * q : S * q + S] = w_ih[rows, :].T
            for u in range(2):
                whh_p[:, u, j, S * q : S * q + S] = w_hh[rows, :][:, hperm[u]].T
            bias_p[0, j, S * q : S * q + S] = bsum[rows]
    ident = np.zeros((128, 32), np.float32)
    for p in range(128):
        ident[p, p % 32] = 1.0
    return wih_p, whh_p, bias_p, ident


def build_nc(T=T_FULL, TC=32, debug=False):
    """Build the per-core Bass program (SPMD: same program on all cores)."""
    _ensure_paths()
    import concourse.bacc as bacc
    import concourse.mybir as mybir
    import concourse.tile as tile
    from contextlib import ExitStack

    fp32 = mybir.dt.float32
    bf16 = mybir.dt.bfloat16
    AF = mybir.ActivationFunctionType

    assert T % TC == 0 and TC % 8 == 0

    nc = bacc.Bacc("TRN2", target_bir_lowering=False, debug=debug)

    x_d = nc.dram_tensor("x", [B, T, I_DIM], fp32, kind="ExternalInput").ap()
    h0_d = nc.dram_tensor("h0", [B, H], bf16, kind="ExternalInput").ap()
    c0_d = nc.dram_tensor("c0", [B, H], fp32, kind="ExternalInput").ap()
    # x/h weights in bf16: matmuls stream at 1 cycle/row at any N and keep the
    # 4-way PE column-group concurrency (fp32 is 2 half-speed passes; fp32r
    # forbids dst partitions != 0, which the column groups need).  The bias
    # round stays fp32/exact.
    wih_d = nc.dram_tensor(
        "wih_p", [I_DIM, NJ, 4 * S], bf16, kind="ExternalInput"
    ).ap()
    whh_d = nc.dram_tensor(
        "whh_p", [128, 2, NJ, 4 * S], bf16, kind="ExternalInput"
    ).ap()
    # bias split b = b1 + b2 with b1 = bf16(b), b2 = bf16(b - b1): two bf16
    # K=1 rounds reproduce the fp32 bias to ~1e-6 while streaming single-pass
    # (the fp32 bias matmuls serialized ~2.7us/step of PE time).
    bias1_d = nc.dram_tensor(
        "bias1_p", [1, NJ, 4 * S], bf16, kind="ExternalInput"
    ).ap()
    bias2_d = nc.dram_tensor(
        "bias2_p", [1, NJ, 4 * S], bf16, kind="ExternalInput"
    ).ap()
    ident_d = nc.dram_tensor("ident", [128, 32], fp32, kind="ExternalInput").ap()
    hn_d = nc.dram_tensor("hn", [B, H], fp32, kind="ExternalOutput").ap()

    with tile.TileContext(nc) as tc, ExitStack() as ctx:
        consts = ctx.enter_context(tc.tile_pool(name="consts", bufs=1))
        states = ctx.enter_context(tc.tile_pool(name="states", bufs=1))
        lhsT_pool = ctx.enter_context(tc.tile_pool(name="lhsT", bufs=3))
        x_pool = ctx.enter_context(tc.tile_pool(name="xstream", bufs=2))
        xT_pool = ctx.enter_context(tc.tile_pool(name="xT", bufs=3))
        ew_pool = ctx.enter_context(tc.tile_pool(name="ew", bufs=3))
        g_psum = ctx.enter_context(tc.tile_pool(name="g_psum", bufs=2, space="PSUM"))
        xt_psum = ctx.enter_context(tc.tile_pool(name="xt_psum", bufs=2, space="PSUM"))

        # ---- constants ----
        wih_sb = consts.tile([I_DIM, NJ, 4 * S], bf16, name="wih_sb")
        nc.sync.dma_start(out=wih_sb, in_=wih_d)
        whh_sb = consts.tile([128, 2, NJ, 4 * S], bf16, name="whh_sb")
        nc.sync.dma_start(out=whh_sb, in_=whh_d)
        bias1_sb = consts.tile([1, NJ, 4 * S], bf16, name="bias1_sb")
        nc.sync.dma_start(out=bias1_sb, in_=bias1_d)
        bias2_sb = consts.tile([1, NJ, 4 * S], bf16, name="bias2_sb")
        nc.sync.dma_start(out=bias2_sb, in_=bias2_d)
        ident_sb = consts.tile([128, 32], fp32, name="ident_sb")
        nc.sync.dma_start(out=ident_sb, in_=ident_d)
        ones_sb = consts.tile([1, 32], bf16, name="ones_sb")
        nc.vector.memset(ones_sb, 1.0)
        # rhs of the zero-contribution "keep the PE p-state warm" matmuls.
        # bf16 so each dummy is one single-cycle-per-row pass (fp32 dummies
        # measured 427ns x 2 passes each -- worse than the cold clock).
        zeros_sb = consts.tile([1, 4 * S], bf16, name="zeros_sb")
        nc.vector.memset(zeros_sb, 0.0)

        # ---- state init (packed) ----
        c_sb = states.tile([128, S], fp32, name="c_sb")
        # h only feeds the gate matmuls (via the transpose), so it lives in
        # bf16; the final step writes a separate fp32 copy for the output.
        h_sb = states.tile([128, S], bf16, name="h_sb")
        for j in range(NJ):
            nc.sync.dma_start(
                out=c_sb[32 * j : 32 * j + 32, :], in_=c0_d[:, S * j : S * j + S]
            )
            nc.sync.dma_start(
                out=h_sb[32 * j : 32 * j + 32, :], in_=h0_d[:, S * j : S * j + S]
            )

        def emit_hT():
            """DVE 32x32 block transpose of packed h -> lhsT column groups.

            hv[32J+y, 32u+x] = h[32J+x, 32u+y] = H[x, 64J+32u+y]; so
            hv[:, 32u:32u+32] is a [K=128, M=32] stationary operand whose
            K-rows enumerate H-inputs in the order 64*(k//32)+32u+(k%32) —
            whh_p is host-permuted to match.
            """
            hT = lhsT_pool.tile([128, 2 * 32], bf16, name="hT")
            nc.vector.transpose(out=hT, in_=h_sb)
            return hT

        hT = emit_hT()

        n_chunks = T // TC

        def fetch(ch):
            """Start the async HBM read of one x chunk (prefetched 1 ahead)."""
            x_sb = x_pool.tile([B, TC, I_DIM], fp32, name="x_sb")
            nc.sync.dma_start(out=x_sb, in_=x_d[:, ch * TC : (ch + 1) * TC, :])
            return x_sb

        def prep_chunk(x_sb):
            """PE-transpose a chunk's x into per-step lhsT tiles."""
            xT_tiles = []
            for g8 in range(TC // 8):
                xt_ps = xt_psum.tile([128, 8 * 32], fp32, name="xt_ps")
                for v in range(8):
                    nc.tensor.transpose(
                        out=xt_ps[:, 32 * v : 32 * v + 32],
                        in_=x_sb[:, g8 * 8 + v, :],
                        identity=ident_sb[0:32, :],
                        tile_position=(0, 0),
                    )
                xT_sb = xT_pool.tile([128, 8 * 32], bf16, name="xT_sb")
                nc.vector.tensor_copy(out=xT_sb, in_=xt_ps)
                xT_tiles.append(xT_sb)
            return xT_tiles

        def start_rounds(xT_sl):
            """Open a step's psum accumulation: bias + x rounds (h-independent,
            so they run on the PE as soon as the bank frees, well before hT)."""
            g_ps = g_psum.tile([128, 4 * S], fp32, name="g_ps")
            for bi, b_sb in enumerate((bias1_sb, bias2_sb)):
                for j in range(NJ):
                    nc.tensor.matmul(
                        g_ps[32 * j : 32 * j + 32, :], ones_sb, b_sb[0:1, j, :],
                        start=(bi == 0), stop=False,
                        tile_position=(0, 32 * j), skip_group_check=True,
                    )
            for j in range(NJ):
                nc.tensor.matmul(
                    g_ps[32 * j : 32 * j + 32, :], xT_sl, wih_sb[:, j, :],
                    start=False, stop=False,
                    tile_position=(0, 32 * j), skip_group_check=True,
                )
            return g_ps

        x_next = fetch(0)
        g_ps = None
        for ch in range(n_chunks):
            x_cur = x_next
            if ch + 1 < n_chunks:
                x_next = fetch(ch + 1)
            xT_tiles = prep_chunk(x_cur)
            if g_ps is None:
                g_ps = start_rounds(xT_tiles[0][:, 0:32])
            for u in range(TC):
                t = ch * TC + u
                # h rounds: the only h_{t-1}-dependent matmuls; round-major
                # across the 4 PE column groups for concurrency.  Column-split
                # i,f first so the sigmoid can start while the o,g half still
                # streams.
                for half in range(2):
                    cols = slice(2 * S * half, 2 * S * half + 2 * S)
                    for rnd in range(2):
                        for j in range(NJ):
                            nc.tensor.matmul(
                                g_ps[32 * j : 32 * j + 32, cols],
                                hT[:, 32 * rnd : 32 * rnd + 32],
                                whh_sb[:, rnd, j, cols],
                                start=False, stop=(rnd == 1),
                                tile_position=(0, 32 * j), skip_group_check=True,
                            )
                # gates: cols [0:64]=i [64:128]=f [128:192]=o [192:256]=g_cell
                sig = ew_pool.tile([128, 3 * S], fp32, name="sig")
                # i,f first: the c-path only needs these + tanh(g); o is
                # consumed much later (h = o*tanh(c)), so its sigmoid runs
                # off the spine while DVE does the c update.
                nc.scalar.activation(
                    sig[:, 0 : 2 * S], g_ps[:, 0 : 2 * S], AF.Sigmoid
                )
                tg = ew_pool.tile([128, S], fp32, name="tg")
                nc.scalar.activation(tg, g_ps[:, 3 * S : 4 * S], AF.Tanh)
                nc.vector.tensor_mul(c_sb, sig[:, S : 2 * S], c_sb)
                t1 = ew_pool.tile([128, S], fp32, name="t1")
                nc.vector.tensor_mul(t1, sig[:, 0:S], tg)
                nc.scalar.activation(
                    sig[:, 2 * S : 3 * S], g_ps[:, 2 * S : 3 * S], AF.Sigmoid
                )
                nc.vector.tensor_add(c_sb, c_sb, t1)
                # bf16: h is bf16 anyway, and bf16 lets tcc serve as the lhsT
                # of the warm-up dummy below.
                tcc = ew_pool.tile([128, S], bf16, name="tcc")
                nc.scalar.activation(tcc, c_sb, AF.Tanh)
                nc.vector.tensor_mul(h_sb, sig[:, 2 * S : 3 * S], tcc)
                if t < T - 1:
                    hT = emit_hT()
                else:
                    # full-precision copy of the final h for the output
                    hf_sb = states.tile([128, S], fp32, name="hf_sb")
                    nc.vector.tensor_mul(hf_sb, sig[:, 2 * S : 3 * S], tcc)
                if u < TC - 1:
                    v = u + 1
                    g_next = start_rounds(
                        xT_tiles[v // 8][:, 32 * (v % 8) : 32 * (v % 8) + 32]
                    )
                    # Zero-contribution bf16 matmuls (out += src_row x 0 = 0)
                    # chained on the tail elementwise tiles: they fire just
                    # before the h rounds, pulling the PE out of its cold
                    # p-state (cold first round measured ~630ns vs ~110 warm).
                    for src in (tcc, h_sb):
                        nc.tensor.matmul(
                            g_next[0:32, 0:S], src[0:1, 0:32], zeros_sb[0:1, 0:S],
                            start=False, stop=False,
                            tile_position=(0, 0), skip_group_check=True,
                        )
                    g_ps = g_next
                else:
                    g_ps = None  # reopened at the next chunk top

        # ---- write back final h (unpack) ----
        for j in range(NJ):
            nc.sync.dma_start(
                out=hn_d[:, S * j : S * j + S], in_=hf_sb[32 * j : 32 * j + 32, :]
            )

    nc.compile()
    return nc


def _shard_inputs(x, h0, c0, w_ih, w_hh, b_ih, b_hh, T=T_FULL):
    import ml_dtypes

    bf16 = ml_dtypes.bfloat16
    wih_p, whh_p, bias_p, ident = _prep_weights(
        np.asarray(w_ih, np.float32),
        np.asarray(w_hh, np.float32),
        np.asarray(b_ih, np.float32),
        np.asarray(b_hh, np.float32),
    )
    wih_p = wih_p.astype(bf16)
    whh_p = whh_p.astype(bf16)
    bias1_p = bias_p.astype(bf16)
    bias2_p = (bias_p - bias1_p.astype(np.float32)).astype(bf16)
    x = np.asarray(x, np.float32)
    h0 = np.asarray(h0, np.float32)
    c0 = np.asarray(c0, np.float32)
    in_maps = []
    for k in range(NCORES):
        bs = slice(B * k, B * (k + 1))
        in_maps.append(
            {
                "x": np.ascontiguousarray(x[bs, :T, :]),
                "h0": np.ascontiguousarray(h0[0, bs, :]).astype(bf16),
                "c0": np.ascontiguousarray(c0[0, bs, :]),
                "wih_p": wih_p,
                "whh_p": whh_p,
                "bias1_p": bias1_p,
                "bias2_p": bias2_p,
                "ident": ident,
            }
        )
    return in_maps


_NC_CACHE = {}


def run_hw(x, h0, c0, w_ih, w_hh, b_ih, b_hh, T=T_FULL, TC=32, trace=False):
    _ensure_paths()
    from concourse.bass_utils import run_bass_kernel_spmd

    key = (T, TC)
    if key not in _NC_CACHE:
        _NC_CACHE[key] = build_nc(T=T, TC=TC)
    nc = _NC_CACHE[key]
    in_maps = _shard_inputs(x, h0, c0, w_ih, w_hh, b_ih, b_hh, T=T)
    res = run_bass_kernel_spmd(nc, in_maps, list(range(NCORES)), trace=trace)
    hn = np.stack([res.results[k]["hn"] for k in range(NCORES)], axis=0)
    return hn.reshape(1, B_TOT, H), res


def kernel(x, h0, c0, w_ih, w_hh, b_ih, b_hh):
    out, _ = run_hw(x, h0, c0, w_ih, w_hh, b_ih, b_hh)
    return out.astype(np.float32)


def _np_reference(x, h0, c0, w_ih, w_hh, b_ih, b_hh, T=None):
    """Numpy oracle for development (matches reference.py)."""
    x = np.asarray(x, np.float64)
    if T is not None:
        x = x[:, :T, :]
    h = np.asarray(h0, np.float64)[0]
    c = np.asarray(c0, np.float64)[0]
    gx = np.einsum("bti,gi->tbg", x, np.asarray(w_ih, np.float64)) + (
        np.asarray(b_ih, np.float64) + np.asarray(b_hh, np.float64)
    )
    W = np.asarray(w_hh, np.float64)

    def sg(v):
        return 1.0 / (1.0 + np.exp(-v))

    for t in range(x.shape[1]):
        g = gx[t] + h @ W.T
        i = sg(g[:, 0:256])
        f = sg(g[:, 256:512])
        gg = np.tanh(g[:, 512:768])
        o = sg(g[:, 768:1024])
        c = f * c + i * gg
        h = o * np.tanh(c)
    return h[None].astype(np.float32)



# revision 39
# speedup vs baseline: 1.7864x; 1.0135x over previous
"""LSTM (single layer, final hidden state) on 8 Trainium2 NeuronCores.

Reference computation (per batch row b):
    gx[t] = x[t] @ w_ih.T + (b_ih + b_hh)
    g     = gx[t] + h @ w_hh.T          # [B, 4H], gate order i,f,g,o
    i,f,o = sigmoid(...), g_c = tanh(...)
    c     = f*c + i*g_c
    h     = o * tanh(c)
returns h after T steps, shape [1, B, H].

Sharding: data-parallel over batch B=256 -> 8 cores x 32. Weights replicated.

Per-core layout ("packed"): partition p = 32*j + b, where j in [0,4) indexes
an H-quarter (H index = 64*j + s, s in [0,64)) and b in [0,32) is the local
batch.  All elementwise tiles are [128, *]:
    c, h            [128, 64]   c[32j+b, s] = C[b, 64j+s]
    gate psum       [128, 256]  cols 64*q+s with q order (i, f, o, g)
Gates are produced by 4 column-tiled concurrent matmuls (tile_position
(0,32j)), accumulating 4 K-rounds: bias (K=1 ones trick), x_t (K=128),
h chunk0 (K=128), h chunk1 (K=128).  The stationary operands are the small
[K,32] transposes of x_t / h, so weight loads are cheap; the big W tiles
stream through the moving port of 4 column groups concurrently.

h -> h.T for the next step is done with 4 concurrent row+col-tiled PE
transposes ([32,64] blocks at tile_position (32j, 64*(j%2))) into one PSUM
tile, then one DVE copy to SBUF.
"""

import os
import sys

import numpy as np

B_TOT, T_FULL, I_DIM, H = 256, 1024, 128, 256
NCORES = 8
B = B_TOT // NCORES  # 32 per core
NJ = 4  # H quarters
S = H // NJ  # 64
# column order within a gate-quarter: (i, f, o, g_cell); row bases in w/b
Q_ROWBASE = (0, 256, 768, 512)


def _ensure_paths():
    for p in ("/opt/trn_rl_repo",):
        if os.path.isdir(p) and p not in sys.path:
            sys.path.append(p)


def _prep_weights(w_ih, w_hh, b_ih, b_hh):
    """Host-side permutation of weights into the packed rhs layouts."""
    wih_p = np.empty((I_DIM, NJ, 4 * S), np.float32)  # [128, 4, 256]
    whh_p = np.empty((128, 2, NJ, 4 * S), np.float32)  # [128, u, j, 256]
    bias_p = np.empty((1, NJ, 4 * S), np.float32)  # [1, 4, 256]
    bsum = (b_ih + b_hh).astype(np.float32)
    # DVE 32x32 block-transpose of packed h puts H-input index
    # 64*(k//32) + 32*u + (k%32) at partition k of lhsT column-group u.
    k = np.arange(128)
    hperm = [64 * (k // 32) + 32 * u + (k % 32) for u in range(2)]
    for q, rb in enumerate(Q_ROWBASE):
        for j in range(NJ):
            rows = slice(rb + S * j, rb + S * j + S)
            wih_p[:, j, S * q : S * q + S] = w_ih[rows, :].T
            for u in range(2):
                whh_p[:, u, j, S * q : S * q + S] = w_hh[rows, :][:, hperm[u]].T
            bias_p[0, j, S * q : S * q + S] = bsum[rows]
    ident = np.zeros((128, 32), np.float32)
    for p in range(128):
        ident[p, p % 32] = 1.0
    return wih_p, whh_p, bias_p, ident


def build_nc(T=T_FULL, TC=32, debug=False):
    """Build the per-core Bass program (SPMD: same program on all cores)."""
    _ensure_paths()
    import concourse.bacc as bacc
    import concourse.mybir as mybir
    import concourse.tile as tile
    from contextlib import ExitStack

    fp32 = mybir.dt.float32
    bf16 = mybir.dt.bfloat16
    AF = mybir.ActivationFunctionType

    assert T % TC == 0 and TC % 8 == 0

    nc = bacc.Bacc("TRN2", target_bir_lowering=False, debug=debug)

    x_d = nc.dram_tensor("x", [B, T, I_DIM], fp32, kind="ExternalInput").ap()
    h0_d = nc.dram_tensor("h0", [B, H], bf16, kind="ExternalInput").ap()
    c0_d = nc.dram_tensor("c0", [B, H], fp32, kind="ExternalInput").ap()
    # x/h weights in bf16: matmuls stream at 1 cycle/row at any N and keep the
    # 4-way PE column-group concurrency (fp32 is 2 half-speed passes; fp32r
    # forbids dst partitions != 0, which the column groups need).  The bias
    # round stays fp32/exact.
    wih_d = nc.dram_tensor(
        "wih_p", [I_DIM, NJ, 4 * S], bf16, kind="ExternalInput"
    ).ap()
    whh_d = nc.dram_tensor(
        "whh_p", [128, 2, NJ, 4 * S], bf16, kind="ExternalInput"
    ).ap()
    # bias split b = b1 + b2 with b1 = bf16(b), b2 = bf16(b - b1): two bf16
    # K=1 rounds reproduce the fp32 bias to ~1e-6 while streaming single-pass
    # (the fp32 bias matmuls serialized ~2.7us/step of PE time).
    bias1_d = nc.dram_tensor(
        "bias1_p", [1, NJ, 4 * S], bf16, kind="ExternalInput"
    ).ap()
    bias2_d = nc.dram_tensor(
        "bias2_p", [1, NJ, 4 * S], bf16, kind="ExternalInput"
    ).ap()
    ident_d = nc.dram_tensor("ident", [128, 32], fp32, kind="ExternalInput").ap()
    hn_d = nc.dram_tensor("hn", [B, H], fp32, kind="ExternalOutput").ap()

    with tile.TileContext(nc) as tc, ExitStack() as ctx:
        consts = ctx.enter_context(tc.tile_pool(name="consts", bufs=1))
        states = ctx.enter_context(tc.tile_pool(name="states", bufs=1))
        lhsT_pool = ctx.enter_context(tc.tile_pool(name="lhsT", bufs=3))
        x_pool = ctx.enter_context(tc.tile_pool(name="xstream", bufs=2))
        xT_pool = ctx.enter_context(tc.tile_pool(name="xT", bufs=3))
        ew_pool = ctx.enter_context(tc.tile_pool(name="ew", bufs=3))
        # bufs=3: with 2, the next step's bias round inherits a WAR dep that
        # resolves only at the CURRENT step's last psum read, pushing it (cold)
        # into the critical window.
        g_psum = ctx.enter_context(tc.tile_pool(name="g_psum", bufs=3, space="PSUM"))
        xt_psum = ctx.enter_context(tc.tile_pool(name="xt_psum", bufs=2, space="PSUM"))

        # ---- constants ----
        wih_sb = consts.tile([I_DIM, NJ, 4 * S], bf16, name="wih_sb")
        nc.sync.dma_start(out=wih_sb, in_=wih_d)
        whh_sb = consts.tile([128, 2, NJ, 4 * S], bf16, name="whh_sb")
        nc.sync.dma_start(out=whh_sb, in_=whh_d)
        bias1_sb = consts.tile([1, NJ, 4 * S], bf16, name="bias1_sb")
        nc.sync.dma_start(out=bias1_sb, in_=bias1_d)
        bias2_sb = consts.tile([1, NJ, 4 * S], bf16, name="bias2_sb")
        nc.sync.dma_start(out=bias2_sb, in_=bias2_d)
        ident_sb = consts.tile([128, 32], fp32, name="ident_sb")
        nc.sync.dma_start(out=ident_sb, in_=ident_d)
        ones_sb = consts.tile([1, 32], bf16, name="ones_sb")
        nc.vector.memset(ones_sb, 1.0)
        # rhs of the zero-contribution "keep the PE p-state warm" matmuls.
        # bf16 so each dummy is one single-cycle-per-row pass (fp32 dummies
        # measured 427ns x 2 passes each -- worse than the cold clock).
        zeros_sb = consts.tile([1, 4 * S], bf16, name="zeros_sb")
        nc.vector.memset(zeros_sb, 0.0)

        # ---- state init (packed) ----
        c_sb = states.tile([128, S], fp32, name="c_sb")
        # h only feeds the gate matmuls (via the transpose), so it lives in
        # bf16; the final step writes a separate fp32 copy for the output.
        h_sb = states.tile([128, S], bf16, name="h_sb")
        for j in range(NJ):
            nc.sync.dma_start(
                out=c_sb[32 * j : 32 * j + 32, :], in_=c0_d[:, S * j : S * j + S]
            )
            nc.sync.dma_start(
                out=h_sb[32 * j : 32 * j + 32, :], in_=h0_d[:, S * j : S * j + S]
            )

        def emit_hT():
            """DVE 32x32 block transpose of packed h -> lhsT column groups.

            hv[32J+y, 32u+x] = h[32J+x, 32u+y] = H[x, 64J+32u+y]; so
            hv[:, 32u:32u+32] is a [K=128, M=32] stationary operand whose
            K-rows enumerate H-inputs in the order 64*(k//32)+32u+(k%32) —
            whh_p is host-permuted to match.
            """
            hT = lhsT_pool.tile([128, 2 * 32], bf16, name="hT")
            nc.vector.transpose(out=hT, in_=h_sb)
            return hT

        hT = emit_hT()

        n_chunks = T // TC

        def fetch(ch):
            """Start the async HBM read of one x chunk (prefetched 1 ahead)."""
            x_sb = x_pool.tile([B, TC, I_DIM], fp32, name="x_sb")
            nc.sync.dma_start(out=x_sb, in_=x_d[:, ch * TC : (ch + 1) * TC, :])
            return x_sb

        def prep_chunk(x_sb):
            """PE-transpose a chunk's x into per-step lhsT tiles."""
            xT_tiles = []
            for g8 in range(TC // 8):
                xt_ps = xt_psum.tile([128, 8 * 32], fp32, name="xt_ps")
                for v in range(8):
                    nc.tensor.transpose(
                        out=xt_ps[:, 32 * v : 32 * v + 32],
                        in_=x_sb[:, g8 * 8 + v, :],
                        identity=ident_sb[0:32, :],
                        tile_position=(0, 0),
                    )
                xT_sb = xT_pool.tile([128, 8 * 32], bf16, name="xT_sb")
                nc.vector.tensor_copy(out=xT_sb, in_=xt_ps)
                xT_tiles.append(xT_sb)
            return xT_tiles

        def start_rounds(xT_sl):
            """Open a step's psum accumulation: bias + x rounds (h-independent,
            so they run on the PE as soon as the bank frees, well before hT)."""
            g_ps = g_psum.tile([128, 4 * S], fp32, name="g_ps")
            for bi, b_sb in enumerate((bias1_sb, bias2_sb)):
                for j in range(NJ):
                    nc.tensor.matmul(
                        g_ps[32 * j : 32 * j + 32, :], ones_sb, b_sb[0:1, j, :],
                        start=(bi == 0), stop=False,
                        tile_position=(0, 32 * j), skip_group_check=True,
                    )
            for j in range(NJ):
                nc.tensor.matmul(
                    g_ps[32 * j : 32 * j + 32, :], xT_sl, wih_sb[:, j, :],
                    start=False, stop=False,
                    tile_position=(0, 32 * j), skip_group_check=True,
                )
            return g_ps

        x_next = fetch(0)
        g_ps = None
        for ch in range(n_chunks):
            x_cur = x_next
            if ch + 1 < n_chunks:
                x_next = fetch(ch + 1)
            xT_tiles = prep_chunk(x_cur)
            if g_ps is None:
                g_ps = start_rounds(xT_tiles[0][:, 0:32])
            for u in range(TC):
                t = ch * TC + u
                # h rounds: the only h_{t-1}-dependent matmuls; round-major
                # across the 4 PE column groups for concurrency.  Column-split
                # i,f first so the sigmoid can start while the o,g half still
                # streams.
                for half in range(2):
                    cols = slice(2 * S * half, 2 * S * half + 2 * S)
                    for rnd in range(2):
                        for j in range(NJ):
                            nc.tensor.matmul(
                                g_ps[32 * j : 32 * j + 32, cols],
                                hT[:, 32 * rnd : 32 * rnd + 32],
                                whh_sb[:, rnd, j, cols],
                                start=False, stop=(rnd == 1),
                                tile_position=(0, 32 * j), skip_group_check=True,
                            )
                # gates: cols [0:64]=i [64:128]=f [128:192]=o [192:256]=g_cell
                # bf16 gate tiles: i*g and o*tanh(c) then hit the DVE 2-byte
                # fast path, and h is bf16 anyway.
                sig = ew_pool.tile([128, 3 * S], bf16, name="sig")
                # i,f first: the c-path only needs these + tanh(g); o is
                # consumed much later (h = o*tanh(c)), so its sigmoid runs
                # off the spine while DVE does the c update.
                nc.scalar.activation(
                    sig[:, 0 : 2 * S], g_ps[:, 0 : 2 * S], AF.Sigmoid
                )
                tg = ew_pool.tile([128, S], bf16, name="tg")
                nc.scalar.activation(tg, g_ps[:, 3 * S : 4 * S], AF.Tanh)
                nc.vector.tensor_mul(c_sb, sig[:, S : 2 * S], c_sb)
                t1 = ew_pool.tile([128, S], bf16, name="t1")
                nc.vector.tensor_mul(t1, sig[:, 0:S], tg)
                nc.scalar.activation(
                    sig[:, 2 * S : 3 * S], g_ps[:, 2 * S : 3 * S], AF.Sigmoid
                )
                nc.vector.tensor_add(c_sb, c_sb, t1)
                # bf16: h is bf16 anyway, and bf16 lets tcc serve as the lhsT
                # of the warm-up dummy below.
                tcc = ew_pool.tile([128, S], bf16, name="tcc")
                nc.scalar.activation(tcc, c_sb, AF.Tanh)
                nc.vector.tensor_mul(h_sb, sig[:, 2 * S : 3 * S], tcc)
                if t < T - 1:
                    hT = emit_hT()
                else:
                    # full-precision copy of the final h for the output
                    hf_sb = states.tile([128, S], fp32, name="hf_sb")
                    nc.vector.tensor_mul(hf_sb, sig[:, 2 * S : 3 * S], tcc)
                if u < TC - 1:
                    v = u + 1
                    g_next = start_rounds(
                        xT_tiles[v // 8][:, 32 * (v % 8) : 32 * (v % 8) + 32]
                    )
                    # Zero-contribution bf16 matmuls (out += src_row x 0 = 0)
                    # chained on the tail elementwise tiles: they fire just
                    # before the h rounds, pulling the PE out of its cold
                    # p-state (cold first round measured ~630ns vs ~110 warm).
                    for src in (tcc, h_sb):
                        nc.tensor.matmul(
                            g_next[0:32, 0:S], src[0:1, 0:32], zeros_sb[0:1, 0:S],
                            start=False, stop=False,
                            tile_position=(0, 0), skip_group_check=True,
                        )
                    g_ps = g_next
                else:
                    g_ps = None  # reopened at the next chunk top

        # ---- write back final h (unpack) ----
        for j in range(NJ):
            nc.sync.dma_start(
                out=hn_d[:, S * j : S * j + S], in_=hf_sb[32 * j : 32 * j + 32, :]
            )

    nc.compile()
    return nc


def _shard_inputs(x, h0, c0, w_ih, w_hh, b_ih, b_hh, T=T_FULL):
    import ml_dtypes

    bf16 = ml_dtypes.bfloat16
    wih_p, whh_p, bias_p, ident = _prep_weights(
        np.asarray(w_ih, np.float32),
        np.asarray(w_hh, np.float32),
        np.asarray(b_ih, np.float32),
        np.asarray(b_hh, np.float32),
    )
    wih_p = wih_p.astype(bf16)
    whh_p = whh_p.astype(bf16)
    bias1_p = bias_p.astype(bf16)
    bias2_p = (bias_p - bias1_p.astype(np.float32)).astype(bf16)
    x = np.asarray(x, np.float32)
    h0 = np.asarray(h0, np.float32)
    c0 = np.asarray(c0, np.float32)
    in_maps = []
    for k in range(NCORES):
        bs = slice(B * k, B * (k + 1))
        in_maps.append(
            {
                "x": np.ascontiguousarray(x[bs, :T, :]),
                "h0": np.ascontiguousarray(h0[0, bs, :]).astype(bf16),
                "c0": np.ascontiguousarray(c0[0, bs, :]),
                "wih_p": wih_p,
                "whh_p": whh_p,
                "bias1_p": bias1_p,
                "bias2_p": bias2_p,
                "ident": ident,
            }
        )
    return in_maps


_NC_CACHE = {}


def run_hw(x, h0, c0, w_ih, w_hh, b_ih, b_hh, T=T_FULL, TC=32, trace=False):
    _ensure_paths()
    from concourse.bass_utils import run_bass_kernel_spmd

    key = (T, TC)
    if key not in _NC_CACHE:
        _NC_CACHE[key] = build_nc(T=T, TC=TC)
    nc = _NC_CACHE[key]
    in_maps = _shard_inputs(x, h0, c0, w_ih, w_hh, b_ih, b_hh, T=T)
    res = run_bass_kernel_spmd(nc, in_maps, list(range(NCORES)), trace=trace)
    hn = np.stack([res.results[k]["hn"] for k in range(NCORES)], axis=0)
    return hn.reshape(1, B_TOT, H), res


def kernel(x, h0, c0, w_ih, w_hh, b_ih, b_hh):
    out, _ = run_hw(x, h0, c0, w_ih, w_hh, b_ih, b_hh)
    return out.astype(np.float32)


def _np_reference(x, h0, c0, w_ih, w_hh, b_ih, b_hh, T=None):
    """Numpy oracle for development (matches reference.py)."""
    x = np.asarray(x, np.float64)
    if T is not None:
        x = x[:, :T, :]
    h = np.asarray(h0, np.float64)[0]
    c = np.asarray(c0, np.float64)[0]
    gx = np.einsum("bti,gi->tbg", x, np.asarray(w_ih, np.float64)) + (
        np.asarray(b_ih, np.float64) + np.asarray(b_hh, np.float64)
    )
    W = np.asarray(w_hh, np.float64)

    def sg(v):
        return 1.0 / (1.0 + np.exp(-v))

    for t in range(x.shape[1]):
        g = gx[t] + h @ W.T
        i = sg(g[:, 0:256])
        f = sg(g[:, 256:512])
        gg = np.tanh(g[:, 512:768])
        o = sg(g[:, 768:1024])
        c = f * c + i * gg
        h = o * np.tanh(c)
    return h[None].astype(np.float32)



# revision 43
# speedup vs baseline: 2.0174x; 1.1294x over previous
"""LSTM (single layer, final hidden state) on 8 Trainium2 NeuronCores.

Reference computation (per batch row b):
    gx[t] = x[t] @ w_ih.T + (b_ih + b_hh)
    g     = gx[t] + h @ w_hh.T          # [B, 4H], gate order i,f,g,o
    i,f,o = sigmoid(...), g_c = tanh(...)
    c     = f*c + i*g_c
    h     = o * tanh(c)
returns h after T steps, shape [1, B, H].

Sharding: data-parallel over batch B=256 -> 8 cores x 32. Weights replicated.

Per-core layout ("packed"): partition p = 32*j + b, where j in [0,4) indexes
an H-quarter (H index = 64*j + s, s in [0,64)) and b in [0,32) is the local
batch.  All elementwise tiles are [128, *]:
    c, h            [128, 64]   c[32j+b, s] = C[b, 64j+s]
    gate psum       [128, 256]  cols 64*q+s with q order (i, f, o, g)
Gates are produced by 4 column-tiled concurrent matmuls (tile_position
(0,32j)), accumulating 4 K-rounds: bias (K=1 ones trick), x_t (K=128),
h chunk0 (K=128), h chunk1 (K=128).  The stationary operands are the small
[K,32] transposes of x_t / h, so weight loads are cheap; the big W tiles
stream through the moving port of 4 column groups concurrently.

h -> h.T for the next step is done with 4 concurrent row+col-tiled PE
transposes ([32,64] blocks at tile_position (32j, 64*(j%2))) into one PSUM
tile, then one DVE copy to SBUF.
"""

import os
import sys

import numpy as np

B_TOT, T_FULL, I_DIM, H = 256, 1024, 128, 256
NCORES = 8
B = B_TOT // NCORES  # 32 per core
NJ = 4  # H quarters
S = H // NJ  # 64
# column order within a gate-quarter: (i, f, o, g_cell); row bases in w/b
Q_ROWBASE = (0, 256, 768, 512)


def _ensure_paths():
    for p in ("/opt/trn_rl_repo",):
        if os.path.isdir(p) and p not in sys.path:
            sys.path.append(p)


def _prep_weights(w_ih, w_hh, b_ih, b_hh):
    """Host-side permutation of weights into the packed rhs layouts."""
    wih_p = np.empty((I_DIM, NJ, 4 * S), np.float32)  # [128, 4, 256]
    whh_p = np.empty((128, 2, NJ, 4 * S), np.float32)  # [128, u, j, 256]
    bias_p = np.empty((1, NJ, 4 * S), np.float32)  # [1, 4, 256]
    bsum = (b_ih + b_hh).astype(np.float32)
    # DVE 32x32 block-transpose of packed h puts H-input index
    # 64*(k//32) + 32*u + (k%32) at partition k of lhsT column-group u.
    k = np.arange(128)
    hperm = [64 * (k // 32) + 32 * u + (k % 32) for u in range(2)]
    for q, rb in enumerate(Q_ROWBASE):
        for j in range(NJ):
            rows = slice(rb + S * j, rb + S * j + S)
            wih_p[:, j, S * q : S * q + S] = w_ih[rows, :].T
            for u in range(2):
                whh_p[:, u, j, S * q : S * q + S] = w_hh[rows, :][:, hperm[u]].T
            bias_p[0, j, S * q : S * q + S] = bsum[rows]
    ident = np.zeros((128, 32), np.float32)
    for p in range(128):
        ident[p, p % 32] = 1.0
    return wih_p, whh_p, bias_p, ident


def build_nc(T=T_FULL, TC=32, debug=False):
    """Build the per-core Bass program (SPMD: same program on all cores)."""
    _ensure_paths()
    import concourse.bacc as bacc
    import concourse.mybir as mybir
    import concourse.tile as tile
    from contextlib import ExitStack

    fp32 = mybir.dt.float32
    bf16 = mybir.dt.bfloat16
    AF = mybir.ActivationFunctionType

    assert T % TC == 0 and TC % 8 == 0

    nc = bacc.Bacc("TRN2", target_bir_lowering=False, debug=debug)

    x_d = nc.dram_tensor("x", [B, T, I_DIM], fp32, kind="ExternalInput").ap()
    h0_d = nc.dram_tensor("h0", [B, H], bf16, kind="ExternalInput").ap()
    c0_d = nc.dram_tensor("c0", [B, H], fp32, kind="ExternalInput").ap()
    # x/h weights in bf16: matmuls stream at 1 cycle/row at any N and keep the
    # 4-way PE column-group concurrency (fp32 is 2 half-speed passes; fp32r
    # forbids dst partitions != 0, which the column groups need).  The bias
    # round stays fp32/exact.
    wih_d = nc.dram_tensor(
        "wih_p", [I_DIM, NJ, 4 * S], bf16, kind="ExternalInput"
    ).ap()
    whh_d = nc.dram_tensor(
        "whh_p", [128, 2, NJ, 4 * S], bf16, kind="ExternalInput"
    ).ap()
    # bias split b = b1 + b2 with b1 = bf16(b), b2 = bf16(b - b1): two bf16
    # K=1 rounds reproduce the fp32 bias to ~1e-6 while streaming single-pass
    # (the fp32 bias matmuls serialized ~2.7us/step of PE time).
    bias1_d = nc.dram_tensor(
        "bias1_p", [1, NJ, 4 * S], bf16, kind="ExternalInput"
    ).ap()
    bias2_d = nc.dram_tensor(
        "bias2_p", [1, NJ, 4 * S], bf16, kind="ExternalInput"
    ).ap()
    ident_d = nc.dram_tensor("ident", [128, 32], fp32, kind="ExternalInput").ap()
    hn_d = nc.dram_tensor("hn", [B, H], fp32, kind="ExternalOutput").ap()

    with tile.TileContext(nc) as tc, ExitStack() as ctx:
        consts = ctx.enter_context(tc.tile_pool(name="consts", bufs=1))
        states = ctx.enter_context(tc.tile_pool(name="states", bufs=1))
        lhsT_pool = ctx.enter_context(tc.tile_pool(name="lhsT", bufs=3))
        x_pool = ctx.enter_context(tc.tile_pool(name="xstream", bufs=2))
        xT_pool = ctx.enter_context(tc.tile_pool(name="xT", bufs=3))
        ew_pool = ctx.enter_context(tc.tile_pool(name="ew", bufs=3))
        # bufs=3: with 2, the next step's bias round inherits a WAR dep that
        # resolves only at the CURRENT step's last psum read, pushing it (cold)
        # into the critical window.
        g_psum = ctx.enter_context(tc.tile_pool(name="g_psum", bufs=3, space="PSUM"))
        xt_psum = ctx.enter_context(tc.tile_pool(name="xt_psum", bufs=2, space="PSUM"))

        # ---- constants ----
        wih_sb = consts.tile([I_DIM, NJ, 4 * S], bf16, name="wih_sb")
        nc.sync.dma_start(out=wih_sb, in_=wih_d)
        whh_sb = consts.tile([128, 2, NJ, 4 * S], bf16, name="whh_sb")
        nc.sync.dma_start(out=whh_sb, in_=whh_d)
        bias1_sb = consts.tile([1, NJ, 4 * S], bf16, name="bias1_sb")
        nc.sync.dma_start(out=bias1_sb, in_=bias1_d)
        bias2_sb = consts.tile([1, NJ, 4 * S], bf16, name="bias2_sb")
        nc.sync.dma_start(out=bias2_sb, in_=bias2_d)
        ident_sb = consts.tile([128, 32], fp32, name="ident_sb")
        nc.sync.dma_start(out=ident_sb, in_=ident_d)
        ones_sb = consts.tile([1, 32], bf16, name="ones_sb")
        nc.vector.memset(ones_sb, 1.0)
        # rhs of the zero-contribution "keep the PE p-state warm" matmuls.
        # bf16 so each dummy is one single-cycle-per-row pass (fp32 dummies
        # measured 427ns x 2 passes each -- worse than the cold clock).
        zeros_sb = consts.tile([1, 4 * S], bf16, name="zeros_sb")
        nc.vector.memset(zeros_sb, 0.0)

        # ---- state init (packed) ----
        c_sb = states.tile([128, S], fp32, name="c_sb")
        # h only feeds the gate matmuls (via the transpose), so it lives in
        # bf16; the final step writes a separate fp32 copy for the output.
        h_sb = states.tile([128, S], bf16, name="h_sb")
        for j in range(NJ):
            nc.sync.dma_start(
                out=c_sb[32 * j : 32 * j + 32, :], in_=c0_d[:, S * j : S * j + S]
            )
            nc.sync.dma_start(
                out=h_sb[32 * j : 32 * j + 32, :], in_=h0_d[:, S * j : S * j + S]
            )

        def emit_hT():
            """DVE 32x32 block transpose of packed h -> lhsT column groups.

            hv[32J+y, 32u+x] = h[32J+x, 32u+y] = H[x, 64J+32u+y]; so
            hv[:, 32u:32u+32] is a [K=128, M=32] stationary operand whose
            K-rows enumerate H-inputs in the order 64*(k//32)+32u+(k%32) —
            whh_p is host-permuted to match.
            """
            hT = lhsT_pool.tile([128, 2 * 32], bf16, name="hT")
            nc.vector.transpose(out=hT, in_=h_sb)
            return hT

        hT = emit_hT()

        n_chunks = T // TC

        def fetch(ch):
            """Start the async HBM read of one x chunk (prefetched 1 ahead)."""
            x_sb = x_pool.tile([B, TC, I_DIM], fp32, name="x_sb")
            nc.sync.dma_start(out=x_sb, in_=x_d[:, ch * TC : (ch + 1) * TC, :])
            return x_sb

        def prep_chunk(x_sb):
            """PE-transpose a chunk's x into per-step lhsT tiles."""
            xT_tiles = []
            for g8 in range(TC // 8):
                xt_ps = xt_psum.tile([128, 8 * 32], fp32, name="xt_ps")
                for v in range(8):
                    nc.tensor.transpose(
                        out=xt_ps[:, 32 * v : 32 * v + 32],
                        in_=x_sb[:, g8 * 8 + v, :],
                        identity=ident_sb[0:32, :],
                        tile_position=(0, 0),
                    )
                xT_sb = xT_pool.tile([128, 8 * 32], bf16, name="xT_sb")
                nc.vector.tensor_copy(out=xT_sb, in_=xt_ps)
                xT_tiles.append(xT_sb)
            return xT_tiles

        def start_rounds(xT_sl):
            """Open a step's psum accumulation: bias + x rounds (h-independent,
            so they run on the PE as soon as the bank frees, well before hT).

            The i,f half and o,g half accumulate in SEPARATE psum tiles so the
            sigmoid's semaphore fires at the i,f stop instead of waiting for
            the whole group."""
            gif = g_psum.tile([128, 2 * S], fp32, name="gif")
            gog = g_psum.tile([128, 2 * S], fp32, name="gog")
            for half, g_ps in enumerate((gif, gog)):
                cols = slice(2 * S * half, 2 * S * half + 2 * S)
                for bi, b_sb in enumerate((bias1_sb, bias2_sb)):
                    for j in range(NJ):
                        nc.tensor.matmul(
                            g_ps[32 * j : 32 * j + 32, :],
                            ones_sb, b_sb[0:1, j, cols],
                            start=(bi == 0), stop=False,
                            tile_position=(0, 32 * j), skip_group_check=True,
                        )
                for j in range(NJ):
                    nc.tensor.matmul(
                        g_ps[32 * j : 32 * j + 32, :], xT_sl, wih_sb[:, j, cols],
                        start=False, stop=False,
                        tile_position=(0, 32 * j), skip_group_check=True,
                    )
            return (gif, gog)

        x_next = fetch(0)
        g_ps = None
        for ch in range(n_chunks):
            x_cur = x_next
            if ch + 1 < n_chunks:
                x_next = fetch(ch + 1)
            xT_tiles = prep_chunk(x_cur)
            if g_ps is None:
                g_ps = start_rounds(xT_tiles[0][:, 0:32])
            for u in range(TC):
                t = ch * TC + u
                # h rounds: the only h_{t-1}-dependent matmuls; round-major
                # across the 4 PE column groups for concurrency.  i,f half
                # first so the sigmoid starts while the o,g half still
                # streams.
                gif, gog = g_ps
                for half, g_half in enumerate((gif, gog)):
                    cols = slice(2 * S * half, 2 * S * half + 2 * S)
                    for rnd in range(2):
                        for j in range(NJ):
                            nc.tensor.matmul(
                                g_half[32 * j : 32 * j + 32, :],
                                hT[:, 32 * rnd : 32 * rnd + 32],
                                whh_sb[:, rnd, j, cols],
                                start=False, stop=(rnd == 1),
                                tile_position=(0, 32 * j), skip_group_check=True,
                            )
                # gates: cols [0:64]=i [64:128]=f [128:192]=o [192:256]=g_cell
                # bf16 gate tiles: i*g and o*tanh(c) then hit the DVE 2-byte
                # fast path, and h is bf16 anyway.
                sig = ew_pool.tile([128, 3 * S], bf16, name="sig")
                # i,f first: the c-path only needs these + tanh(g); o is
                # consumed much later (h = o*tanh(c)), so its sigmoid runs
                # off the spine while DVE does the c update.
                nc.scalar.activation(sig[:, 0 : 2 * S], gif, AF.Sigmoid)
                tg = ew_pool.tile([128, S], bf16, name="tg")
                nc.scalar.activation(tg, gog[:, S : 2 * S], AF.Tanh)
                nc.vector.tensor_mul(c_sb, sig[:, S : 2 * S], c_sb)
                t1 = ew_pool.tile([128, S], bf16, name="t1")
                nc.vector.tensor_mul(t1, sig[:, 0:S], tg)
                nc.scalar.activation(
                    sig[:, 2 * S : 3 * S], gog[:, 0:S], AF.Sigmoid
                )
                nc.vector.tensor_add(c_sb, c_sb, t1)
                # bf16: h is bf16 anyway, and bf16 lets tcc serve as the lhsT
                # of the warm-up dummy below.
                tcc = ew_pool.tile([128, S], bf16, name="tcc")
                nc.scalar.activation(tcc, c_sb, AF.Tanh)
                nc.vector.tensor_mul(h_sb, sig[:, 2 * S : 3 * S], tcc)
                if t < T - 1:
                    hT = emit_hT()
                else:
                    # full-precision copy of the final h for the output
                    hf_sb = states.tile([128, S], fp32, name="hf_sb")
                    nc.vector.tensor_mul(hf_sb, sig[:, 2 * S : 3 * S], tcc)
                if u < TC - 1:
                    v = u + 1
                    g_next = start_rounds(
                        xT_tiles[v // 8][:, 32 * (v % 8) : 32 * (v % 8) + 32]
                    )
                    # Zero-contribution bf16 matmuls (out += src_row x 0 = 0)
                    # chained on the tail elementwise tiles: they fire just
                    # before the h rounds, pulling the PE out of its cold
                    # p-state (cold first round measured ~630ns vs ~110 warm).
                    for src in (tcc, h_sb):
                        nc.tensor.matmul(
                            g_next[0][0:32, 0:S], src[0:1, 0:32],
                            zeros_sb[0:1, 0:S],
                            start=False, stop=False,
                            tile_position=(0, 0), skip_group_check=True,
                        )
                    g_ps = g_next
                else:
                    g_ps = None  # reopened at the next chunk top

        # ---- write back final h (unpack) ----
        for j in range(NJ):
            nc.sync.dma_start(
                out=hn_d[:, S * j : S * j + S], in_=hf_sb[32 * j : 32 * j + 32, :]
            )

    nc.compile()
    return nc


def _shard_inputs(x, h0, c0, w_ih, w_hh, b_ih, b_hh, T=T_FULL):
    import ml_dtypes

    bf16 = ml_dtypes.bfloat16
    wih_p, whh_p, bias_p, ident = _prep_weights(
        np.asarray(w_ih, np.float32),
        np.asarray(w_hh, np.float32),
        np.asarray(b_ih, np.float32),
        np.asarray(b_hh, np.float32),
    )
    wih_p = wih_p.astype(bf16)
    whh_p = whh_p.astype(bf16)
    bias1_p = bias_p.astype(bf16)
    bias2_p = (bias_p - bias1_p.astype(np.float32)).astype(bf16)
    x = np.asarray(x, np.float32)
    h0 = np.asarray(h0, np.float32)
    c0 = np.asarray(c0, np.float32)
    in_maps = []
    for k in range(NCORES):
        bs = slice(B * k, B * (k + 1))
        in_maps.append(
            {
                "x": np.ascontiguousarray(x[bs, :T, :]),
                "h0": np.ascontiguousarray(h0[0, bs, :]).astype(bf16),
                "c0": np.ascontiguousarray(c0[0, bs, :]),
                "wih_p": wih_p,
                "whh_p": whh_p,
                "bias1_p": bias1_p,
                "bias2_p": bias2_p,
                "ident": ident,
            }
        )
    return in_maps


_NC_CACHE = {}


def run_hw(x, h0, c0, w_ih, w_hh, b_ih, b_hh, T=T_FULL, TC=32, trace=False):
    _ensure_paths()
    from concourse.bass_utils import run_bass_kernel_spmd

    key = (T, TC)
    if key not in _NC_CACHE:
        _NC_CACHE[key] = build_nc(T=T, TC=TC)
    nc = _NC_CACHE[key]
    in_maps = _shard_inputs(x, h0, c0, w_ih, w_hh, b_ih, b_hh, T=T)
    res = run_bass_kernel_spmd(nc, in_maps, list(range(NCORES)), trace=trace)
    hn = np.stack([res.results[k]["hn"] for k in range(NCORES)], axis=0)
    return hn.reshape(1, B_TOT, H), res


def kernel(x, h0, c0, w_ih, w_hh, b_ih, b_hh):
    out, _ = run_hw(x, h0, c0, w_ih, w_hh, b_ih, b_hh)
    return out.astype(np.float32)


def _np_reference(x, h0, c0, w_ih, w_hh, b_ih, b_hh, T=None):
    """Numpy oracle for development (matches reference.py)."""
    x = np.asarray(x, np.float64)
    if T is not None:
        x = x[:, :T, :]
    h = np.asarray(h0, np.float64)[0]
    c = np.asarray(c0, np.float64)[0]
    gx = np.einsum("bti,gi->tbg", x, np.asarray(w_ih, np.float64)) + (
        np.asarray(b_ih, np.float64) + np.asarray(b_hh, np.float64)
    )
    W = np.asarray(w_hh, np.float64)

    def sg(v):
        return 1.0 / (1.0 + np.exp(-v))

    for t in range(x.shape[1]):
        g = gx[t] + h @ W.T
        i = sg(g[:, 0:256])
        f = sg(g[:, 256:512])
        gg = np.tanh(g[:, 512:768])
        o = sg(g[:, 768:1024])
        c = f * c + i * gg
        h = o * np.tanh(c)
    return h[None].astype(np.float32)



# revision 44
# speedup vs baseline: 2.0192x; 1.0009x over previous
"""LSTM (single layer, final hidden state) on 8 Trainium2 NeuronCores.

Reference computation (per batch row b):
    gx[t] = x[t] @ w_ih.T + (b_ih + b_hh)
    g     = gx[t] + h @ w_hh.T          # [B, 4H], gate order i,f,g,o
    i,f,o = sigmoid(...), g_c = tanh(...)
    c     = f*c + i*g_c
    h     = o * tanh(c)
returns h after T steps, shape [1, B, H].

Sharding: data-parallel over batch B=256 -> 8 cores x 32. Weights replicated.

Per-core layout ("packed"): partition p = 32*j + b, where j in [0,4) indexes
an H-quarter (H index = 64*j + s, s in [0,64)) and b in [0,32) is the local
batch.  All elementwise tiles are [128, *]:
    c, h            [128, 64]   c[32j+b, s] = C[b, 64j+s]
    gate psum       [128, 256]  cols 64*q+s with q order (i, f, o, g)
Gates are produced by 4 column-tiled concurrent matmuls (tile_position
(0,32j)); the stationary operands are the small [K,32] transposes of
x_t / h, the W tiles stream through the moving port of the 4 column
groups concurrently.  h -> h.T is one DVE 32x32-block stream transpose.

The wall time is T x the per-step dependency-chain latency
(hT -> h-matmul -> sigmoid/tanh -> c update -> tanh(c) -> o*tanh(c) ->
transpose -> next matmul), so everything h-independent is pulled off that
spine and the spine ops are minimized:
  * x/h matmuls and weights in bf16 (1 cycle/row at any N; fp32 runs two
    half-rate passes, fp32r forbids the column-group dst partitions).
  * bias = bf16(b) + bf16(b - bf16(b)): two single-pass bf16 K=1 rounds
    reproduce the fp32 bias to ~1e-6 (one fp32 bias round serialized
    ~2.7us/step of PE time).
  * bias + x rounds for step t+1 are opened during step t's elementwise
    window (own psum tiles, bufs=3 so no WAR dep drags them onto the
    spine).
  * The i,f gate half and the o,g half accumulate in separate psum tiles:
    the sigmoid's semaphore fires at the i,f stop instead of the full
    group; sigmoid(o) runs off-spine after tanh(g).
  * Two zero-contribution bf16 matmuls chained on tcc / h keep the PE
    p-state warm through the elementwise window (cold first matmul
    measured ~630ns vs ~110ns warm).
  * Gate tiles in bf16 for the DVE 2-byte fast path; c stays fp32, and
    the final step writes a separate fp32 h for the output.

Measured on trn2: 4,529,865 ns (all-fp32 baseline) -> 2,803,748 ns,
rel err 6.6e-3 (gate: 2e-2).
"""

import os
import sys

import numpy as np

B_TOT, T_FULL, I_DIM, H = 256, 1024, 128, 256
NCORES = 8
B = B_TOT // NCORES  # 32 per core
NJ = 4  # H quarters
S = H // NJ  # 64
# column order within a gate-quarter: (i, f, o, g_cell); row bases in w/b
Q_ROWBASE = (0, 256, 768, 512)


def _ensure_paths():
    for p in ("/opt/trn_rl_repo",):
        if os.path.isdir(p) and p not in sys.path:
            sys.path.append(p)


def _prep_weights(w_ih, w_hh, b_ih, b_hh):
    """Host-side permutation of weights into the packed rhs layouts."""
    wih_p = np.empty((I_DIM, NJ, 4 * S), np.float32)  # [128, 4, 256]
    whh_p = np.empty((128, 2, NJ, 4 * S), np.float32)  # [128, u, j, 256]
    bias_p = np.empty((1, NJ, 4 * S), np.float32)  # [1, 4, 256]
    bsum = (b_ih + b_hh).astype(np.float32)
    # DVE 32x32 block-transpose of packed h puts H-input index
    # 64*(k//32) + 32*u + (k%32) at partition k of lhsT column-group u.
    k = np.arange(128)
    hperm = [64 * (k // 32) + 32 * u + (k % 32) for u in range(2)]
    for q, rb in enumerate(Q_ROWBASE):
        for j in range(NJ):
            rows = slice(rb + S * j, rb + S * j + S)
            wih_p[:, j, S * q : S * q + S] = w_ih[rows, :].T
            for u in range(2):
                whh_p[:, u, j, S * q : S * q + S] = w_hh[rows, :][:, hperm[u]].T
            bias_p[0, j, S * q : S * q + S] = bsum[rows]
    ident = np.zeros((128, 32), np.float32)
    for p in range(128):
        ident[p, p % 32] = 1.0
    return wih_p, whh_p, bias_p, ident


def build_nc(T=T_FULL, TC=32, debug=False):
    """Build the per-core Bass program (SPMD: same program on all cores)."""
    _ensure_paths()
    import concourse.bacc as bacc
    import concourse.mybir as mybir
    import concourse.tile as tile
    from contextlib import ExitStack

    fp32 = mybir.dt.float32
    bf16 = mybir.dt.bfloat16
    AF = mybir.ActivationFunctionType

    assert T % TC == 0 and TC % 8 == 0

    nc = bacc.Bacc("TRN2", target_bir_lowering=False, debug=debug)

    x_d = nc.dram_tensor("x", [B, T, I_DIM], fp32, kind="ExternalInput").ap()
    h0_d = nc.dram_tensor("h0", [B, H], bf16, kind="ExternalInput").ap()
    c0_d = nc.dram_tensor("c0", [B, H], fp32, kind="ExternalInput").ap()
    # x/h weights in bf16: matmuls stream at 1 cycle/row at any N and keep the
    # 4-way PE column-group concurrency (fp32 is 2 half-speed passes; fp32r
    # forbids dst partitions != 0, which the column groups need).  The bias
    # round stays fp32/exact.
    wih_d = nc.dram_tensor(
        "wih_p", [I_DIM, NJ, 4 * S], bf16, kind="ExternalInput"
    ).ap()
    whh_d = nc.dram_tensor(
        "whh_p", [128, 2, NJ, 4 * S], bf16, kind="ExternalInput"
    ).ap()
    # bias split b = b1 + b2 with b1 = bf16(b), b2 = bf16(b - b1): two bf16
    # K=1 rounds reproduce the fp32 bias to ~1e-6 while streaming single-pass
    # (the fp32 bias matmuls serialized ~2.7us/step of PE time).
    bias1_d = nc.dram_tensor(
        "bias1_p", [1, NJ, 4 * S], bf16, kind="ExternalInput"
    ).ap()
    bias2_d = nc.dram_tensor(
        "bias2_p", [1, NJ, 4 * S], bf16, kind="ExternalInput"
    ).ap()
    ident_d = nc.dram_tensor("ident", [128, 32], fp32, kind="ExternalInput").ap()
    hn_d = nc.dram_tensor("hn", [B, H], fp32, kind="ExternalOutput").ap()

    with tile.TileContext(nc) as tc, ExitStack() as ctx:
        consts = ctx.enter_context(tc.tile_pool(name="consts", bufs=1))
        states = ctx.enter_context(tc.tile_pool(name="states", bufs=1))
        lhsT_pool = ctx.enter_context(tc.tile_pool(name="lhsT", bufs=3))
        x_pool = ctx.enter_context(tc.tile_pool(name="xstream", bufs=2))
        xT_pool = ctx.enter_context(tc.tile_pool(name="xT", bufs=3))
        ew_pool = ctx.enter_context(tc.tile_pool(name="ew", bufs=3))
        # bufs=3: with 2, the next step's bias round inherits a WAR dep that
        # resolves only at the CURRENT step's last psum read, pushing it (cold)
        # into the critical window.
        g_psum = ctx.enter_context(tc.tile_pool(name="g_psum", bufs=3, space="PSUM"))
        xt_psum = ctx.enter_context(tc.tile_pool(name="xt_psum", bufs=2, space="PSUM"))

        # ---- constants ----
        wih_sb = consts.tile([I_DIM, NJ, 4 * S], bf16, name="wih_sb")
        nc.sync.dma_start(out=wih_sb, in_=wih_d)
        whh_sb = consts.tile([128, 2, NJ, 4 * S], bf16, name="whh_sb")
        nc.sync.dma_start(out=whh_sb, in_=whh_d)
        bias1_sb = consts.tile([1, NJ, 4 * S], bf16, name="bias1_sb")
        nc.sync.dma_start(out=bias1_sb, in_=bias1_d)
        bias2_sb = consts.tile([1, NJ, 4 * S], bf16, name="bias2_sb")
        nc.sync.dma_start(out=bias2_sb, in_=bias2_d)
        ident_sb = consts.tile([128, 32], fp32, name="ident_sb")
        nc.sync.dma_start(out=ident_sb, in_=ident_d)
        ones_sb = consts.tile([1, 32], bf16, name="ones_sb")
        nc.vector.memset(ones_sb, 1.0)
        # rhs of the zero-contribution "keep the PE p-state warm" matmuls.
        # bf16 so each dummy is one single-cycle-per-row pass (fp32 dummies
        # measured 427ns x 2 passes each -- worse than the cold clock).
        zeros_sb = consts.tile([1, 4 * S], bf16, name="zeros_sb")
        nc.vector.memset(zeros_sb, 0.0)

        # ---- state init (packed) ----
        c_sb = states.tile([128, S], fp32, name="c_sb")
        # h only feeds the gate matmuls (via the transpose), so it lives in
        # bf16; the final step writes a separate fp32 copy for the output.
        h_sb = states.tile([128, S], bf16, name="h_sb")
        for j in range(NJ):
            nc.sync.dma_start(
                out=c_sb[32 * j : 32 * j + 32, :], in_=c0_d[:, S * j : S * j + S]
            )
            nc.sync.dma_start(
                out=h_sb[32 * j : 32 * j + 32, :], in_=h0_d[:, S * j : S * j + S]
            )

        def emit_hT():
            """DVE 32x32 block transpose of packed h -> lhsT column groups.

            hv[32J+y, 32u+x] = h[32J+x, 32u+y] = H[x, 64J+32u+y]; so
            hv[:, 32u:32u+32] is a [K=128, M=32] stationary operand whose
            K-rows enumerate H-inputs in the order 64*(k//32)+32u+(k%32) —
            whh_p is host-permuted to match.
            """
            hT = lhsT_pool.tile([128, 2 * 32], bf16, name="hT")
            nc.vector.transpose(out=hT, in_=h_sb)
            return hT

        hT = emit_hT()

        n_chunks = T // TC

        def fetch(ch):
            """Start the async HBM read of one x chunk (prefetched 1 ahead)."""
            x_sb = x_pool.tile([B, TC, I_DIM], fp32, name="x_sb")
            nc.sync.dma_start(out=x_sb, in_=x_d[:, ch * TC : (ch + 1) * TC, :])
            return x_sb

        def prep_chunk(x_sb):
            """PE-transpose a chunk's x into per-step lhsT tiles."""
            xT_tiles = []
            for g8 in range(TC // 8):
                xt_ps = xt_psum.tile([128, 8 * 32], fp32, name="xt_ps")
                for v in range(8):
                    nc.tensor.transpose(
                        out=xt_ps[:, 32 * v : 32 * v + 32],
                        in_=x_sb[:, g8 * 8 + v, :],
                        identity=ident_sb[0:32, :],
                        tile_position=(0, 0),
                    )
                xT_sb = xT_pool.tile([128, 8 * 32], bf16, name="xT_sb")
                nc.vector.tensor_copy(out=xT_sb, in_=xt_ps)
                xT_tiles.append(xT_sb)
            return xT_tiles

        def start_rounds(xT_sl):
            """Open a step's psum accumulation: bias + x rounds (h-independent,
            so they run on the PE as soon as the bank frees, well before hT).

            The i,f half and o,g half accumulate in SEPARATE psum tiles so the
            sigmoid's semaphore fires at the i,f stop instead of waiting for
            the whole group."""
            gif = g_psum.tile([128, 2 * S], fp32, name="gif")
            gog = g_psum.tile([128, 2 * S], fp32, name="gog")
            for half, g_ps in enumerate((gif, gog)):
                cols = slice(2 * S * half, 2 * S * half + 2 * S)
                for bi, b_sb in enumerate((bias1_sb, bias2_sb)):
                    for j in range(NJ):
                        nc.tensor.matmul(
                            g_ps[32 * j : 32 * j + 32, :],
                            ones_sb, b_sb[0:1, j, cols],
                            start=(bi == 0), stop=False,
                            tile_position=(0, 32 * j), skip_group_check=True,
                        )
                for j in range(NJ):
                    nc.tensor.matmul(
                        g_ps[32 * j : 32 * j + 32, :], xT_sl, wih_sb[:, j, cols],
                        start=False, stop=False,
                        tile_position=(0, 32 * j), skip_group_check=True,
                    )
            return (gif, gog)

        x_next = fetch(0)
        g_ps = None
        for ch in range(n_chunks):
            x_cur = x_next
            if ch + 1 < n_chunks:
                x_next = fetch(ch + 1)
            xT_tiles = prep_chunk(x_cur)
            if g_ps is None:
                g_ps = start_rounds(xT_tiles[0][:, 0:32])
            for u in range(TC):
                t = ch * TC + u
                # h rounds: the only h_{t-1}-dependent matmuls; round-major
                # across the 4 PE column groups for concurrency.  i,f half
                # first so the sigmoid starts while the o,g half still
                # streams.
                gif, gog = g_ps
                for half, g_half in enumerate((gif, gog)):
                    cols = slice(2 * S * half, 2 * S * half + 2 * S)
                    for rnd in range(2):
                        for j in range(NJ):
                            nc.tensor.matmul(
                                g_half[32 * j : 32 * j + 32, :],
                                hT[:, 32 * rnd : 32 * rnd + 32],
                                whh_sb[:, rnd, j, cols],
                                start=False, stop=(rnd == 1),
                                tile_position=(0, 32 * j), skip_group_check=True,
                            )
                # gates: cols [0:64]=i [64:128]=f [128:192]=o [192:256]=g_cell
                # bf16 gate tiles: i*g and o*tanh(c) then hit the DVE 2-byte
                # fast path, and h is bf16 anyway.
                sig = ew_pool.tile([128, 3 * S], bf16, name="sig")
                # i,f first: the c-path only needs these + tanh(g); o is
                # consumed much later (h = o*tanh(c)), so its sigmoid runs
                # off the spine while DVE does the c update.
                nc.scalar.activation(sig[:, 0 : 2 * S], gif, AF.Sigmoid)
                tg = ew_pool.tile([128, S], bf16, name="tg")
                nc.scalar.activation(tg, gog[:, S : 2 * S], AF.Tanh)
                nc.vector.tensor_mul(c_sb, sig[:, S : 2 * S], c_sb)
                t1 = ew_pool.tile([128, S], bf16, name="t1")
                nc.vector.tensor_mul(t1, sig[:, 0:S], tg)
                nc.scalar.activation(
                    sig[:, 2 * S : 3 * S], gog[:, 0:S], AF.Sigmoid
                )
                nc.vector.tensor_add(c_sb, c_sb, t1)
                # bf16: h is bf16 anyway, and bf16 lets tcc serve as the lhsT
                # of the warm-up dummy below.
                tcc = ew_pool.tile([128, S], bf16, name="tcc")
                nc.scalar.activation(tcc, c_sb, AF.Tanh)
                nc.vector.tensor_mul(h_sb, sig[:, 2 * S : 3 * S], tcc)
                if t < T - 1:
                    hT = emit_hT()
                else:
                    # full-precision copy of the final h for the output
                    hf_sb = states.tile([128, S], fp32, name="hf_sb")
                    nc.vector.tensor_mul(hf_sb, sig[:, 2 * S : 3 * S], tcc)
                if u < TC - 1:
                    v = u + 1
                    g_next = start_rounds(
                        xT_tiles[v // 8][:, 32 * (v % 8) : 32 * (v % 8) + 32]
                    )
                    # Zero-contribution bf16 matmuls (out += src_row x 0 = 0)
                    # chained on the tail elementwise tiles: they fire just
                    # before the h rounds, pulling the PE out of its cold
                    # p-state (cold first round measured ~630ns vs ~110 warm).
                    for src in (tcc, h_sb):
                        nc.tensor.matmul(
                            g_next[0][0:32, 0:S], src[0:1, 0:32],
                            zeros_sb[0:1, 0:S],
                            start=False, stop=False,
                            tile_position=(0, 0), skip_group_check=True,
                        )
                    g_ps = g_next
                else:
                    g_ps = None  # reopened at the next chunk top

        # ---- write back final h (unpack) ----
        for j in range(NJ):
            nc.sync.dma_start(
                out=hn_d[:, S * j : S * j + S], in_=hf_sb[32 * j : 32 * j + 32, :]
            )

    nc.compile()
    return nc


def _shard_inputs(x, h0, c0, w_ih, w_hh, b_ih, b_hh, T=T_FULL):
    import ml_dtypes

    bf16 = ml_dtypes.bfloat16
    wih_p, whh_p, bias_p, ident = _prep_weights(
        np.asarray(w_ih, np.float32),
        np.asarray(w_hh, np.float32),
        np.asarray(b_ih, np.float32),
        np.asarray(b_hh, np.float32),
    )
    wih_p = wih_p.astype(bf16)
    whh_p = whh_p.astype(bf16)
    bias1_p = bias_p.astype(bf16)
    bias2_p = (bias_p - bias1_p.astype(np.float32)).astype(bf16)
    x = np.asarray(x, np.float32)
    h0 = np.asarray(h0, np.float32)
    c0 = np.asarray(c0, np.float32)
    in_maps = []
    for k in range(NCORES):
        bs = slice(B * k, B * (k + 1))
        in_maps.append(
            {
                "x": np.ascontiguousarray(x[bs, :T, :]),
                "h0": np.ascontiguousarray(h0[0, bs, :]).astype(bf16),
                "c0": np.ascontiguousarray(c0[0, bs, :]),
                "wih_p": wih_p,
                "whh_p": whh_p,
                "bias1_p": bias1_p,
                "bias2_p": bias2_p,
                "ident": ident,
            }
        )
    return in_maps


_NC_CACHE = {}


def run_hw(x, h0, c0, w_ih, w_hh, b_ih, b_hh, T=T_FULL, TC=32, trace=False):
    _ensure_paths()
    from concourse.bass_utils import run_bass_kernel_spmd

    key = (T, TC)
    if key not in _NC_CACHE:
        _NC_CACHE[key] = build_nc(T=T, TC=TC)
    nc = _NC_CACHE[key]
    in_maps = _shard_inputs(x, h0, c0, w_ih, w_hh, b_ih, b_hh, T=T)
    res = run_bass_kernel_spmd(nc, in_maps, list(range(NCORES)), trace=trace)
    hn = np.stack([res.results[k]["hn"] for k in range(NCORES)], axis=0)
    return hn.reshape(1, B_TOT, H), res


def kernel(x, h0, c0, w_ih, w_hh, b_ih, b_hh):
    out, _ = run_hw(x, h0, c0, w_ih, w_hh, b_ih, b_hh)
    return out.astype(np.float32)


def _np_reference(x, h0, c0, w_ih, w_hh, b_ih, b_hh, T=None):
    """Numpy oracle for development (matches reference.py)."""
    x = np.asarray(x, np.float64)
    if T is not None:
        x = x[:, :T, :]
    h = np.asarray(h0, np.float64)[0]
    c = np.asarray(c0, np.float64)[0]
    gx = np.einsum("bti,gi->tbg", x, np.asarray(w_ih, np.float64)) + (
        np.asarray(b_ih, np.float64) + np.asarray(b_hh, np.float64)
    )
    W = np.asarray(w_hh, np.float64)

    def sg(v):
        return 1.0 / (1.0 + np.exp(-v))

    for t in range(x.shape[1]):
        g = gx[t] + h @ W.T
        i = sg(g[:, 0:256])
        f = sg(g[:, 256:512])
        gg = np.tanh(g[:, 512:768])
        o = sg(g[:, 768:1024])
        c = f * c + i * gg
        h = o * np.tanh(c)
    return h[None].astype(np.float32)



# revision 45
# speedup vs baseline: 2.0214x; 1.0011x over previous
"""LSTM (single layer, final hidden state) on 8 Trainium2 NeuronCores.

Reference computation (per batch row b):
    gx[t] = x[t] @ w_ih.T + (b_ih + b_hh)
    g     = gx[t] + h @ w_hh.T          # [B, 4H], gate order i,f,g,o
    i,f,o = sigmoid(...), g_c = tanh(...)
    c     = f*c + i*g_c
    h     = o * tanh(c)
returns h after T steps, shape [1, B, H].

Sharding: data-parallel over batch B=256 -> 8 cores x 32. Weights replicated.

Per-core layout ("packed"): partition p = 32*j + b, where j in [0,4) indexes
an H-quarter (H index = 64*j + s, s in [0,64)) and b in [0,32) is the local
batch.  All elementwise tiles are [128, *]:
    c, h            [128, 64]   c[32j+b, s] = C[b, 64j+s]
    gate psum       [128, 256]  cols 64*q+s with q order (i, f, o, g)
Gates are produced by 4 column-tiled concurrent matmuls (tile_position
(0,32j)); the stationary operands are the small [K,32] transposes of
x_t / h, the W tiles stream through the moving port of the 4 column
groups concurrently.  h -> h.T is one DVE 32x32-block stream transpose.

The wall time is T x the per-step dependency-chain latency
(hT -> h-matmul -> sigmoid/tanh -> c update -> tanh(c) -> o*tanh(c) ->
transpose -> next matmul), so everything h-independent is pulled off that
spine and the spine ops are minimized:
  * x/h matmuls and weights in bf16 (1 cycle/row at any N; fp32 runs two
    half-rate passes, fp32r forbids the column-group dst partitions).
  * bias = bf16(b) + bf16(b - bf16(b)): two single-pass bf16 K=1 rounds
    reproduce the fp32 bias to ~1e-6 (one fp32 bias round serialized
    ~2.7us/step of PE time).
  * bias + x rounds for step t+1 are opened during step t's elementwise
    window (own psum tiles, bufs=3 so no WAR dep drags them onto the
    spine).
  * The i,f gate half and the o,g half accumulate in separate psum tiles:
    the sigmoid's semaphore fires at the i,f stop instead of the full
    group; sigmoid(o) runs off-spine after tanh(g).
  * Two zero-contribution bf16 matmuls chained on tcc / h keep the PE
    p-state warm through the elementwise window (cold first matmul
    measured ~630ns vs ~110ns warm).
  * Gate tiles in bf16 for the DVE 2-byte fast path; c stays fp32, and
    the final step writes a separate fp32 h for the output.

Measured on trn2: 4,529,865 ns (all-fp32 baseline) -> 2,803,748 ns,
rel err 6.6e-3 (gate: 2e-2).
"""

import os
import sys

import numpy as np

B_TOT, T_FULL, I_DIM, H = 256, 1024, 128, 256
NCORES = 8
B = B_TOT // NCORES  # 32 per core
NJ = 4  # H quarters
S = H // NJ  # 64
# column order within a gate-quarter: (i, f, o, g_cell); row bases in w/b
Q_ROWBASE = (0, 256, 768, 512)


def _ensure_paths():
    for p in ("/opt/trn_rl_repo",):
        if os.path.isdir(p) and p not in sys.path:
            sys.path.append(p)


def _prep_weights(w_ih, w_hh, b_ih, b_hh):
    """Host-side permutation of weights into the packed rhs layouts."""
    wih_p = np.empty((I_DIM, NJ, 4 * S), np.float32)  # [128, 4, 256]
    whh_p = np.empty((128, 2, NJ, 4 * S), np.float32)  # [128, u, j, 256]
    bias_p = np.empty((1, NJ, 4 * S), np.float32)  # [1, 4, 256]
    bsum = (b_ih + b_hh).astype(np.float32)
    # DVE 32x32 block-transpose of packed h puts H-input index
    # 64*(k//32) + 32*u + (k%32) at partition k of lhsT column-group u.
    k = np.arange(128)
    hperm = [64 * (k // 32) + 32 * u + (k % 32) for u in range(2)]
    for q, rb in enumerate(Q_ROWBASE):
        for j in range(NJ):
            rows = slice(rb + S * j, rb + S * j + S)
            wih_p[:, j, S * q : S * q + S] = w_ih[rows, :].T
            for u in range(2):
                whh_p[:, u, j, S * q : S * q + S] = w_hh[rows, :][:, hperm[u]].T
            bias_p[0, j, S * q : S * q + S] = bsum[rows]
    ident = np.zeros((128, 32), np.float32)
    for p in range(128):
        ident[p, p % 32] = 1.0
    return wih_p, whh_p, bias_p, ident


def build_nc(T=T_FULL, TC=32, debug=False):
    """Build the per-core Bass program (SPMD: same program on all cores)."""
    _ensure_paths()
    import concourse.bacc as bacc
    import concourse.mybir as mybir
    import concourse.tile as tile
    from contextlib import ExitStack

    fp32 = mybir.dt.float32
    bf16 = mybir.dt.bfloat16
    AF = mybir.ActivationFunctionType

    assert T % TC == 0 and TC % 8 == 0

    nc = bacc.Bacc("TRN2", target_bir_lowering=False, debug=debug)

    x_d = nc.dram_tensor("x", [B, T, I_DIM], fp32, kind="ExternalInput").ap()
    h0_d = nc.dram_tensor("h0", [B, H], bf16, kind="ExternalInput").ap()
    c0_d = nc.dram_tensor("c0", [B, H], fp32, kind="ExternalInput").ap()
    # x/h weights in bf16: matmuls stream at 1 cycle/row at any N and keep the
    # 4-way PE column-group concurrency (fp32 is 2 half-speed passes; fp32r
    # forbids dst partitions != 0, which the column groups need).  The bias
    # round stays fp32/exact.
    wih_d = nc.dram_tensor(
        "wih_p", [I_DIM, NJ, 4 * S], bf16, kind="ExternalInput"
    ).ap()
    whh_d = nc.dram_tensor(
        "whh_p", [128, 2, NJ, 4 * S], bf16, kind="ExternalInput"
    ).ap()
    # bias split b = b1 + b2 with b1 = bf16(b), b2 = bf16(b - b1): two bf16
    # K=1 rounds reproduce the fp32 bias to ~1e-6 while streaming single-pass
    # (the fp32 bias matmuls serialized ~2.7us/step of PE time).
    bias1_d = nc.dram_tensor(
        "bias1_p", [1, NJ, 4 * S], bf16, kind="ExternalInput"
    ).ap()
    bias2_d = nc.dram_tensor(
        "bias2_p", [1, NJ, 4 * S], bf16, kind="ExternalInput"
    ).ap()
    ident_d = nc.dram_tensor("ident", [128, 32], fp32, kind="ExternalInput").ap()
    hn_d = nc.dram_tensor("hn", [B, H], fp32, kind="ExternalOutput").ap()

    with tile.TileContext(nc) as tc, ExitStack() as ctx:
        consts = ctx.enter_context(tc.tile_pool(name="consts", bufs=1))
        states = ctx.enter_context(tc.tile_pool(name="states", bufs=1))
        lhsT_pool = ctx.enter_context(tc.tile_pool(name="lhsT", bufs=3))
        x_pool = ctx.enter_context(tc.tile_pool(name="xstream", bufs=2))
        xT_pool = ctx.enter_context(tc.tile_pool(name="xT", bufs=3))
        ew_pool = ctx.enter_context(tc.tile_pool(name="ew", bufs=3))
        # bufs=3: with 2, the next step's bias round inherits a WAR dep that
        # resolves only at the CURRENT step's last psum read, pushing it (cold)
        # into the critical window.
        g_psum = ctx.enter_context(tc.tile_pool(name="g_psum", bufs=3, space="PSUM"))
        xt_psum = ctx.enter_context(tc.tile_pool(name="xt_psum", bufs=2, space="PSUM"))

        # ---- constants ----
        wih_sb = consts.tile([I_DIM, NJ, 4 * S], bf16, name="wih_sb")
        nc.sync.dma_start(out=wih_sb, in_=wih_d)
        whh_sb = consts.tile([128, 2, NJ, 4 * S], bf16, name="whh_sb")
        nc.sync.dma_start(out=whh_sb, in_=whh_d)
        bias1_sb = consts.tile([1, NJ, 4 * S], bf16, name="bias1_sb")
        nc.sync.dma_start(out=bias1_sb, in_=bias1_d)
        bias2_sb = consts.tile([1, NJ, 4 * S], bf16, name="bias2_sb")
        nc.sync.dma_start(out=bias2_sb, in_=bias2_d)
        ident_sb = consts.tile([128, 32], fp32, name="ident_sb")
        nc.sync.dma_start(out=ident_sb, in_=ident_d)
        ones_sb = consts.tile([1, 32], bf16, name="ones_sb")
        nc.vector.memset(ones_sb, 1.0)
        # rhs of the zero-contribution "keep the PE p-state warm" matmuls.
        # bf16 so each dummy is one single-cycle-per-row pass (fp32 dummies
        # measured 427ns x 2 passes each -- worse than the cold clock).
        zeros_sb = consts.tile([1, 4 * S], bf16, name="zeros_sb")
        nc.vector.memset(zeros_sb, 0.0)

        # ---- state init (packed) ----
        c_sb = states.tile([128, S], fp32, name="c_sb")
        # h only feeds the gate matmuls (via the transpose), so it lives in
        # bf16; the final step writes a separate fp32 copy for the output.
        h_sb = states.tile([128, S], bf16, name="h_sb")
        for j in range(NJ):
            nc.sync.dma_start(
                out=c_sb[32 * j : 32 * j + 32, :], in_=c0_d[:, S * j : S * j + S]
            )
            nc.sync.dma_start(
                out=h_sb[32 * j : 32 * j + 32, :], in_=h0_d[:, S * j : S * j + S]
            )

        def emit_hT():
            """DVE 32x32 block transpose of packed h -> lhsT column groups.

            hv[32J+y, 32u+x] = h[32J+x, 32u+y] = H[x, 64J+32u+y]; so
            hv[:, 32u:32u+32] is a [K=128, M=32] stationary operand whose
            K-rows enumerate H-inputs in the order 64*(k//32)+32u+(k%32) —
            whh_p is host-permuted to match.
            """
            hT = lhsT_pool.tile([128, 2 * 32], bf16, name="hT")
            nc.vector.transpose(out=hT, in_=h_sb)
            return hT

        hT = emit_hT()

        n_chunks = T // TC

        def fetch(ch):
            """Start the async HBM read of one x chunk (prefetched 1 ahead)."""
            x_sb = x_pool.tile([B, TC, I_DIM], fp32, name="x_sb")
            nc.sync.dma_start(out=x_sb, in_=x_d[:, ch * TC : (ch + 1) * TC, :])
            return x_sb

        def prep_chunk(x_sb):
            """PE-transpose a chunk's x into per-step lhsT tiles."""
            xT_tiles = []
            for g8 in range(TC // 8):
                xt_ps = xt_psum.tile([128, 8 * 32], fp32, name="xt_ps")
                for v in range(8):
                    nc.tensor.transpose(
                        out=xt_ps[:, 32 * v : 32 * v + 32],
                        in_=x_sb[:, g8 * 8 + v, :],
                        identity=ident_sb[0:32, :],
                        tile_position=(0, 0),
                    )
                xT_sb = xT_pool.tile([128, 8 * 32], bf16, name="xT_sb")
                nc.vector.tensor_copy(out=xT_sb, in_=xt_ps)
                xT_tiles.append(xT_sb)
            return xT_tiles

        def start_rounds(xT_sl):
            """Open a step's psum accumulation: bias + x rounds (h-independent,
            so they run on the PE as soon as the bank frees, well before hT).

            The i,f half and o,g half accumulate in SEPARATE psum tiles so the
            sigmoid's semaphore fires at the i,f stop instead of waiting for
            the whole group."""
            gif = g_psum.tile([128, 2 * S], fp32, name="gif")
            gog = g_psum.tile([128, 2 * S], fp32, name="gog")
            for half, g_ps in enumerate((gif, gog)):
                cols = slice(2 * S * half, 2 * S * half + 2 * S)
                for bi, b_sb in enumerate((bias1_sb, bias2_sb)):
                    for j in range(NJ):
                        nc.tensor.matmul(
                            g_ps[32 * j : 32 * j + 32, :],
                            ones_sb, b_sb[0:1, j, cols],
                            start=(bi == 0), stop=False,
                            tile_position=(0, 32 * j), skip_group_check=True,
                        )
                for j in range(NJ):
                    nc.tensor.matmul(
                        g_ps[32 * j : 32 * j + 32, :], xT_sl, wih_sb[:, j, cols],
                        start=False, stop=False,
                        tile_position=(0, 32 * j), skip_group_check=True,
                    )
            return (gif, gog)

        x_next = fetch(0)
        g_ps = None
        for ch in range(n_chunks):
            x_cur = x_next
            if ch + 1 < n_chunks:
                x_next = fetch(ch + 1)
            xT_tiles = prep_chunk(x_cur)
            if g_ps is None:
                g_ps = start_rounds(xT_tiles[0][:, 0:32])
            for u in range(TC):
                t = ch * TC + u
                # h rounds: the only h_{t-1}-dependent matmuls; round-major
                # across the 4 PE column groups for concurrency.  i,f half
                # first so the sigmoid starts while the o,g half still
                # streams.
                gif, gog = g_ps
                for half, g_half in enumerate((gif, gog)):
                    cols = slice(2 * S * half, 2 * S * half + 2 * S)
                    for rnd in range(2):
                        for j in range(NJ):
                            nc.tensor.matmul(
                                g_half[32 * j : 32 * j + 32, :],
                                hT[:, 32 * rnd : 32 * rnd + 32],
                                whh_sb[:, rnd, j, cols],
                                start=False, stop=(rnd == 1),
                                tile_position=(0, 32 * j), skip_group_check=True,
                            )
                # gates: cols [0:64]=i [64:128]=f [128:192]=o [192:256]=g_cell
                # bf16 gate tiles: i*g and o*tanh(c) then hit the DVE 2-byte
                # fast path, and h is bf16 anyway.
                sig = ew_pool.tile([128, 3 * S], bf16, name="sig")
                # i,f first: the c-path only needs these + tanh(g); o is
                # consumed much later (h = o*tanh(c)), so its sigmoid runs
                # off the spine while DVE does the c update.
                nc.scalar.activation(sig[:, 0 : 2 * S], gif, AF.Sigmoid)
                tg = ew_pool.tile([128, S], bf16, name="tg")
                nc.scalar.activation(tg, gog[:, S : 2 * S], AF.Tanh)
                nc.vector.tensor_mul(c_sb, sig[:, S : 2 * S], c_sb)
                t1 = ew_pool.tile([128, S], bf16, name="t1")
                nc.vector.tensor_mul(t1, sig[:, 0:S], tg)
                nc.scalar.activation(
                    sig[:, 2 * S : 3 * S], gog[:, 0:S], AF.Sigmoid
                )
                nc.vector.tensor_add(c_sb, c_sb, t1)
                # bf16: h is bf16 anyway, and bf16 lets tcc serve as the lhsT
                # of the warm-up dummy below.
                tcc = ew_pool.tile([128, S], bf16, name="tcc")
                nc.scalar.activation(tcc, c_sb, AF.Tanh)
                nc.vector.tensor_mul(h_sb, sig[:, 2 * S : 3 * S], tcc)
                if t < T - 1:
                    hT = emit_hT()
                else:
                    # full-precision copy of the final h for the output
                    hf_sb = states.tile([128, S], fp32, name="hf_sb")
                    nc.vector.tensor_mul(hf_sb, sig[:, 2 * S : 3 * S], tcc)
                if u < TC - 1:
                    v = u + 1
                    g_next = start_rounds(
                        xT_tiles[v // 8][:, 32 * (v % 8) : 32 * (v % 8) + 32]
                    )
                    # Zero-contribution bf16 matmul (out += src_row x 0 = 0)
                    # chained on tanh(c): it fires during the h = o*tanh(c) /
                    # transpose window, pulling the PE out of its cold p-state
                    # (cold first round measured ~630ns vs ~110 warm).  A
                    # second dummy on h_sb fired so late it sat in the
                    # in-order PE queue ahead of the h rounds and delayed
                    # them by its full duration.
                    nc.tensor.matmul(
                        g_next[0][0:32, 0:S], tcc[0:1, 0:32],
                        zeros_sb[0:1, 0:S],
                        start=False, stop=False,
                        tile_position=(0, 0), skip_group_check=True,
                    )
                    g_ps = g_next
                else:
                    g_ps = None  # reopened at the next chunk top

        # ---- write back final h (unpack) ----
        for j in range(NJ):
            nc.sync.dma_start(
                out=hn_d[:, S * j : S * j + S], in_=hf_sb[32 * j : 32 * j + 32, :]
            )

    nc.compile()
    return nc


def _shard_inputs(x, h0, c0, w_ih, w_hh, b_ih, b_hh, T=T_FULL):
    import ml_dtypes

    bf16 = ml_dtypes.bfloat16
    wih_p, whh_p, bias_p, ident = _prep_weights(
        np.asarray(w_ih, np.float32),
        np.asarray(w_hh, np.float32),
        np.asarray(b_ih, np.float32),
        np.asarray(b_hh, np.float32),
    )
    wih_p = wih_p.astype(bf16)
    whh_p = whh_p.astype(bf16)
    bias1_p = bias_p.astype(bf16)
    bias2_p = (bias_p - bias1_p.astype(np.float32)).astype(bf16)
    x = np.asarray(x, np.float32)
    h0 = np.asarray(h0, np.float32)
    c0 = np.asarray(c0, np.float32)
    in_maps = []
    for k in range(NCORES):
        bs = slice(B * k, B * (k + 1))
        in_maps.append(
            {
                "x": np.ascontiguousarray(x[bs, :T, :]),
                "h0": np.ascontiguousarray(h0[0, bs, :]).astype(bf16),
                "c0": np.ascontiguousarray(c0[0, bs, :]),
                "wih_p": wih_p,
                "whh_p": whh_p,
                "bias1_p": bias1_p,
                "bias2_p": bias2_p,
                "ident": ident,
            }
        )
    return in_maps


_NC_CACHE = {}


def run_hw(x, h0, c0, w_ih, w_hh, b_ih, b_hh, T=T_FULL, TC=32, trace=False):
    _ensure_paths()
    from concourse.bass_utils import run_bass_kernel_spmd

    key = (T, TC)
    if key not in _NC_CACHE:
        _NC_CACHE[key] = build_nc(T=T, TC=TC)
    nc = _NC_CACHE[key]
    in_maps = _shard_inputs(x, h0, c0, w_ih, w_hh, b_ih, b_hh, T=T)
    res = run_bass_kernel_spmd(nc, in_maps, list(range(NCORES)), trace=trace)
    hn = np.stack([res.results[k]["hn"] for k in range(NCORES)], axis=0)
    return hn.reshape(1, B_TOT, H), res


def kernel(x, h0, c0, w_ih, w_hh, b_ih, b_hh):
    out, _ = run_hw(x, h0, c0, w_ih, w_hh, b_ih, b_hh)
    return out.astype(np.float32)


def _np_reference(x, h0, c0, w_ih, w_hh, b_ih, b_hh, T=None):
    """Numpy oracle for development (matches reference.py)."""
    x = np.asarray(x, np.float64)
    if T is not None:
        x = x[:, :T, :]
    h = np.asarray(h0, np.float64)[0]
    c = np.asarray(c0, np.float64)[0]
    gx = np.einsum("bti,gi->tbg", x, np.asarray(w_ih, np.float64)) + (
        np.asarray(b_ih, np.float64) + np.asarray(b_hh, np.float64)
    )
    W = np.asarray(w_hh, np.float64)

    def sg(v):
        return 1.0 / (1.0 + np.exp(-v))

    for t in range(x.shape[1]):
        g = gx[t] + h @ W.T
        i = sg(g[:, 0:256])
        f = sg(g[:, 256:512])
        gg = np.tanh(g[:, 512:768])
        o = sg(g[:, 768:1024])
        c = f * c + i * gg
        h = o * np.tanh(c)
    return h[None].astype(np.float32)



# revision 52
# speedup vs baseline: 2.0249x; 1.0017x over previous
"""LSTM (single layer, final hidden state) on 8 Trainium2 NeuronCores.

Reference computation (per batch row b):
    gx[t] = x[t] @ w_ih.T + (b_ih + b_hh)
    g     = gx[t] + h @ w_hh.T          # [B, 4H], gate order i,f,g,o
    i,f,o = sigmoid(...), g_c = tanh(...)
    c     = f*c + i*g_c
    h     = o * tanh(c)
returns h after T steps, shape [1, B, H].

Sharding: data-parallel over batch B=256 -> 8 cores x 32. Weights replicated.

Per-core layout ("packed"): partition p = 32*j + b, where j in [0,4) indexes
an H-quarter (H index = 64*j + s, s in [0,64)) and b in [0,32) is the local
batch.  All elementwise tiles are [128, *]:
    c, h            [128, 64]   c[32j+b, s] = C[b, 64j+s]
    gate psum       [128, 256]  cols 64*q+s with q order (i, f, o, g)
Gates are produced by 4 column-tiled concurrent matmuls (tile_position
(0,32j)); the stationary operands are the small [K,32] transposes of
x_t / h, the W tiles stream through the moving port of the 4 column
groups concurrently.  h -> h.T is one DVE 32x32-block stream transpose.

The wall time is T x the per-step dependency-chain latency
(hT -> h-matmul -> sigmoid/tanh -> c update -> tanh(c) -> o*tanh(c) ->
transpose -> next matmul), so everything h-independent is pulled off that
spine and the spine ops are minimized:
  * x/h matmuls and weights in bf16 (1 cycle/row at any N; fp32 runs two
    half-rate passes, fp32r forbids the column-group dst partitions).
  * bias = bf16(b) + bf16(b - bf16(b)): two single-pass bf16 K=1 rounds
    reproduce the fp32 bias to ~1e-6 (one fp32 bias round serialized
    ~2.7us/step of PE time).
  * bias + x rounds for step t+1 are opened during step t's elementwise
    window (own psum tiles, bufs=3 so no WAR dep drags them onto the
    spine).
  * The i,f gate half and the o,g half accumulate in separate psum tiles:
    the sigmoid's semaphore fires at the i,f stop instead of the full
    group; sigmoid(o) runs off-spine after tanh(g).
  * Two zero-contribution bf16 matmuls chained on tcc / h keep the PE
    p-state warm through the elementwise window (cold first matmul
    measured ~630ns vs ~110ns warm).
  * Gate tiles in bf16 for the DVE 2-byte fast path; c stays fp32, and
    the final step writes a separate fp32 h for the output.

Measured on trn2: 4,529,865 ns (all-fp32 baseline) -> 2,803,748 ns,
rel err 6.6e-3 (gate: 2e-2).
"""

import os
import sys

import numpy as np

B_TOT, T_FULL, I_DIM, H = 256, 1024, 128, 256
NCORES = 8
B = B_TOT // NCORES  # 32 per core
NJ = 4  # H quarters
S = H // NJ  # 64
# column order within a gate-quarter: (i, f, o, g_cell); row bases in w/b
Q_ROWBASE = (0, 256, 768, 512)


def _ensure_paths():
    for p in ("/opt/trn_rl_repo",):
        if os.path.isdir(p) and p not in sys.path:
            sys.path.append(p)


def _prep_weights(w_ih, w_hh, b_ih, b_hh):
    """Host-side permutation of weights into the packed rhs layouts."""
    wih_p = np.empty((I_DIM, NJ, 4 * S), np.float32)  # [128, 4, 256]
    whh_p = np.empty((128, 2, NJ, 4 * S), np.float32)  # [128, u, j, 256]
    bias_p = np.empty((1, NJ, 4 * S), np.float32)  # [1, 4, 256]
    bsum = (b_ih + b_hh).astype(np.float32)
    # DVE 32x32 block-transpose of packed h puts H-input index
    # 64*(k//32) + 32*u + (k%32) at partition k of lhsT column-group u.
    k = np.arange(128)
    hperm = [64 * (k // 32) + 32 * u + (k % 32) for u in range(2)]
    for q, rb in enumerate(Q_ROWBASE):
        for j in range(NJ):
            rows = slice(rb + S * j, rb + S * j + S)
            wih_p[:, j, S * q : S * q + S] = w_ih[rows, :].T
            for u in range(2):
                whh_p[:, u, j, S * q : S * q + S] = w_hh[rows, :][:, hperm[u]].T
            bias_p[0, j, S * q : S * q + S] = bsum[rows]
    ident = np.zeros((128, 32), np.float32)
    for p in range(128):
        ident[p, p % 32] = 1.0
    return wih_p, whh_p, bias_p, ident


def build_nc(T=T_FULL, TC=32, debug=False):
    """Build the per-core Bass program (SPMD: same program on all cores)."""
    _ensure_paths()
    import concourse.bacc as bacc
    import concourse.mybir as mybir
    import concourse.tile as tile
    from contextlib import ExitStack

    fp32 = mybir.dt.float32
    bf16 = mybir.dt.bfloat16
    AF = mybir.ActivationFunctionType

    assert T % TC == 0 and TC % 8 == 0

    nc = bacc.Bacc("TRN2", target_bir_lowering=False, debug=debug)

    x_d = nc.dram_tensor("x", [B, T, I_DIM], fp32, kind="ExternalInput").ap()
    h0_d = nc.dram_tensor("h0", [B, H], bf16, kind="ExternalInput").ap()
    c0_d = nc.dram_tensor("c0", [B, H], fp32, kind="ExternalInput").ap()
    # x/h weights in bf16: matmuls stream at 1 cycle/row at any N and keep the
    # 4-way PE column-group concurrency (fp32 is 2 half-speed passes; fp32r
    # forbids dst partitions != 0, which the column groups need).  The bias
    # round stays fp32/exact.
    wih_d = nc.dram_tensor(
        "wih_p", [I_DIM, NJ, 4 * S], bf16, kind="ExternalInput"
    ).ap()
    whh_d = nc.dram_tensor(
        "whh_p", [128, 2, NJ, 4 * S], bf16, kind="ExternalInput"
    ).ap()
    # bias split b = b1 + b2 with b1 = bf16(b), b2 = bf16(b - b1): two bf16
    # K=1 rounds reproduce the fp32 bias to ~1e-6 while streaming single-pass
    # (the fp32 bias matmuls serialized ~2.7us/step of PE time).
    bias1_d = nc.dram_tensor(
        "bias1_p", [1, NJ, 4 * S], bf16, kind="ExternalInput"
    ).ap()
    bias2_d = nc.dram_tensor(
        "bias2_p", [1, NJ, 4 * S], bf16, kind="ExternalInput"
    ).ap()
    ident_d = nc.dram_tensor("ident", [128, 32], fp32, kind="ExternalInput").ap()
    hn_d = nc.dram_tensor("hn", [B, H], fp32, kind="ExternalOutput").ap()

    with tile.TileContext(nc) as tc, ExitStack() as ctx:
        consts = ctx.enter_context(tc.tile_pool(name="consts", bufs=1))
        states = ctx.enter_context(tc.tile_pool(name="states", bufs=1))
        lhsT_pool = ctx.enter_context(tc.tile_pool(name="lhsT", bufs=3))
        x_pool = ctx.enter_context(tc.tile_pool(name="xstream", bufs=2))
        xT_pool = ctx.enter_context(tc.tile_pool(name="xT", bufs=3))
        ew_pool = ctx.enter_context(tc.tile_pool(name="ew", bufs=3))
        # bufs=3: with 2, the next step's bias round inherits a WAR dep that
        # resolves only at the CURRENT step's last psum read, pushing it (cold)
        # into the critical window.
        g_psum = ctx.enter_context(tc.tile_pool(name="g_psum", bufs=3, space="PSUM"))
        # xt bufs=1 frees a PSUM bank for the c state (transposes are
        # off-spine, amortized over 32 steps).
        xt_psum = ctx.enter_context(tc.tile_pool(name="xt_psum", bufs=1, space="PSUM"))
        c_psum = ctx.enter_context(tc.tile_pool(name="c_psum", bufs=1, space="PSUM"))

        # ---- constants ----
        wih_sb = consts.tile([I_DIM, NJ, 4 * S], bf16, name="wih_sb")
        nc.sync.dma_start(out=wih_sb, in_=wih_d)
        whh_sb = consts.tile([128, 2, NJ, 4 * S], bf16, name="whh_sb")
        nc.sync.dma_start(out=whh_sb, in_=whh_d)
        bias1_sb = consts.tile([1, NJ, 4 * S], bf16, name="bias1_sb")
        nc.sync.dma_start(out=bias1_sb, in_=bias1_d)
        bias2_sb = consts.tile([1, NJ, 4 * S], bf16, name="bias2_sb")
        nc.sync.dma_start(out=bias2_sb, in_=bias2_d)
        ident_sb = consts.tile([128, 32], fp32, name="ident_sb")
        nc.sync.dma_start(out=ident_sb, in_=ident_d)
        ones_sb = consts.tile([1, 32], bf16, name="ones_sb")
        nc.vector.memset(ones_sb, 1.0)
        # rhs of the zero-contribution "keep the PE p-state warm" matmuls.
        # bf16 so each dummy is one single-cycle-per-row pass (fp32 dummies
        # measured 427ns x 2 passes each -- worse than the cold clock).
        zeros_sb = consts.tile([1, 4 * S], bf16, name="zeros_sb")
        nc.vector.memset(zeros_sb, 0.0)

        # ---- state init (packed) ----
        # c lives in PSUM: ACT reads PSUM ~126ns faster than SBUF, which puts
        # tanh(c) on the faster path every step.
        c_init = states.tile([128, S], fp32, name="c_init")
        c_sb = c_psum.tile([128, S], fp32, name="c_ps")
        # h only feeds the gate matmuls (via the transpose), so it lives in
        # bf16; the final step writes a separate fp32 copy for the output.
        h_sb = states.tile([128, S], bf16, name="h_sb")
        for j in range(NJ):
            nc.sync.dma_start(
                out=c_init[32 * j : 32 * j + 32, :], in_=c0_d[:, S * j : S * j + S]
            )
            nc.sync.dma_start(
                out=h_sb[32 * j : 32 * j + 32, :], in_=h0_d[:, S * j : S * j + S]
            )
        nc.vector.tensor_copy(out=c_sb, in_=c_init)

        def emit_hT():
            """DVE 32x32 block transpose of packed h -> lhsT column groups.

            hv[32J+y, 32u+x] = h[32J+x, 32u+y] = H[x, 64J+32u+y]; so
            hv[:, 32u:32u+32] is a [K=128, M=32] stationary operand whose
            K-rows enumerate H-inputs in the order 64*(k//32)+32u+(k%32) —
            whh_p is host-permuted to match.
            """
            hT = lhsT_pool.tile([128, 2 * 32], bf16, name="hT")
            nc.vector.transpose(out=hT, in_=h_sb)
            return hT

        hT = emit_hT()

        n_chunks = T // TC

        def fetch(ch):
            """Start the async HBM read of one x chunk (prefetched 1 ahead)."""
            x_sb = x_pool.tile([B, TC, I_DIM], fp32, name="x_sb")
            nc.sync.dma_start(out=x_sb, in_=x_d[:, ch * TC : (ch + 1) * TC, :])
            return x_sb

        def prep_chunk(x_sb):
            """PE-transpose a chunk's x into per-step lhsT tiles."""
            xT_tiles = []
            for g8 in range(TC // 8):
                xt_ps = xt_psum.tile([128, 8 * 32], fp32, name="xt_ps")
                for v in range(8):
                    nc.tensor.transpose(
                        out=xt_ps[:, 32 * v : 32 * v + 32],
                        in_=x_sb[:, g8 * 8 + v, :],
                        identity=ident_sb[0:32, :],
                        tile_position=(0, 0),
                    )
                xT_sb = xT_pool.tile([128, 8 * 32], bf16, name="xT_sb")
                nc.vector.tensor_copy(out=xT_sb, in_=xt_ps)
                xT_tiles.append(xT_sb)
            return xT_tiles

        def start_rounds(xT_sl):
            """Open a step's psum accumulation: bias + x rounds (h-independent,
            so they run on the PE as soon as the bank frees, well before hT).

            The i,f half and o,g half accumulate in SEPARATE psum tiles so the
            sigmoid's semaphore fires at the i,f stop instead of waiting for
            the whole group."""
            gif = g_psum.tile([128, 2 * S], fp32, name="gif")
            gog = g_psum.tile([128, 2 * S], fp32, name="gog")
            for half, g_ps in enumerate((gif, gog)):
                cols = slice(2 * S * half, 2 * S * half + 2 * S)
                for bi, b_sb in enumerate((bias1_sb, bias2_sb)):
                    for j in range(NJ):
                        nc.tensor.matmul(
                            g_ps[32 * j : 32 * j + 32, :],
                            ones_sb, b_sb[0:1, j, cols],
                            start=(bi == 0), stop=False,
                            tile_position=(0, 32 * j), skip_group_check=True,
                        )
                for j in range(NJ):
                    nc.tensor.matmul(
                        g_ps[32 * j : 32 * j + 32, :], xT_sl, wih_sb[:, j, cols],
                        start=False, stop=False,
                        tile_position=(0, 32 * j), skip_group_check=True,
                    )
            return (gif, gog)

        x_next = fetch(0)
        g_ps = None
        for ch in range(n_chunks):
            x_cur = x_next
            if ch + 1 < n_chunks:
                x_next = fetch(ch + 1)
            xT_tiles = prep_chunk(x_cur)
            if g_ps is None:
                g_ps = start_rounds(xT_tiles[0][:, 0:32])
            for u in range(TC):
                t = ch * TC + u
                # h rounds: the only h_{t-1}-dependent matmuls; round-major
                # across the 4 PE column groups for concurrency.  i,f half
                # first so the sigmoid starts while the o,g half still
                # streams.
                gif, gog = g_ps
                for half, g_half in enumerate((gif, gog)):
                    cols = slice(2 * S * half, 2 * S * half + 2 * S)
                    for rnd in range(2):
                        for j in range(NJ):
                            nc.tensor.matmul(
                                g_half[32 * j : 32 * j + 32, :],
                                hT[:, 32 * rnd : 32 * rnd + 32],
                                whh_sb[:, rnd, j, cols],
                                start=False, stop=(rnd == 1),
                                tile_position=(0, 32 * j), skip_group_check=True,
                            )
                # gates: cols [0:64]=i [64:128]=f [128:192]=o [192:256]=g_cell
                # bf16 gate tiles: i*g and o*tanh(c) then hit the DVE 2-byte
                # fast path, and h is bf16 anyway.
                sig = ew_pool.tile([128, 3 * S], bf16, name="sig")
                # i,f first: the c-path only needs these + tanh(g); o is
                # consumed much later (h = o*tanh(c)), so its sigmoid runs
                # off the spine while DVE does the c update.
                nc.scalar.activation(sig[:, 0 : 2 * S], gif, AF.Sigmoid)
                tg = ew_pool.tile([128, S], bf16, name="tg")
                nc.scalar.activation(tg, gog[:, S : 2 * S], AF.Tanh)
                nc.vector.tensor_mul(c_sb, sig[:, S : 2 * S], c_sb)
                t1 = ew_pool.tile([128, S], bf16, name="t1")
                nc.vector.tensor_mul(t1, sig[:, 0:S], tg)
                nc.scalar.activation(
                    sig[:, 2 * S : 3 * S], gog[:, 0:S], AF.Sigmoid
                )
                nc.vector.tensor_add(c_sb, c_sb, t1)
                # bf16: h is bf16 anyway, and bf16 lets tcc serve as the lhsT
                # of the warm-up dummy below.
                tcc = ew_pool.tile([128, S], bf16, name="tcc")
                nc.scalar.activation(tcc, c_sb, AF.Tanh)
                nc.vector.tensor_mul(h_sb, sig[:, 2 * S : 3 * S], tcc)
                if t < T - 1:
                    hT = emit_hT()
                else:
                    # full-precision copy of the final h for the output
                    hf_sb = states.tile([128, S], fp32, name="hf_sb")
                    nc.vector.tensor_mul(hf_sb, sig[:, 2 * S : 3 * S], tcc)
                if u < TC - 1:
                    v = u + 1
                    g_next = start_rounds(
                        xT_tiles[v // 8][:, 32 * (v % 8) : 32 * (v % 8) + 32]
                    )
                    # Zero-contribution bf16 matmul (out += src_row x 0 = 0)
                    # chained on tanh(c): it fires during the h = o*tanh(c) /
                    # transpose window, pulling the PE out of its cold p-state
                    # (cold first round measured ~630ns vs ~110 warm).  A
                    # second dummy on h_sb fired so late it sat in the
                    # in-order PE queue ahead of the h rounds and delayed
                    # them by its full duration.
                    nc.tensor.matmul(
                        g_next[0][0:32, 0:S], tcc[0:1, 0:32],
                        zeros_sb[0:1, 0:S],
                        start=False, stop=False,
                        tile_position=(0, 0), skip_group_check=True,
                    )
                    g_ps = g_next
                else:
                    g_ps = None  # reopened at the next chunk top

        # ---- write back final h (unpack) ----
        for j in range(NJ):
            nc.sync.dma_start(
                out=hn_d[:, S * j : S * j + S], in_=hf_sb[32 * j : 32 * j + 32, :]
            )

    nc.compile()
    return nc


def _shard_inputs(x, h0, c0, w_ih, w_hh, b_ih, b_hh, T=T_FULL):
    import ml_dtypes

    bf16 = ml_dtypes.bfloat16
    wih_p, whh_p, bias_p, ident = _prep_weights(
        np.asarray(w_ih, np.float32),
        np.asarray(w_hh, np.float32),
        np.asarray(b_ih, np.float32),
        np.asarray(b_hh, np.float32),
    )
    wih_p = wih_p.astype(bf16)
    whh_p = whh_p.astype(bf16)
    bias1_p = bias_p.astype(bf16)
    bias2_p = (bias_p - bias1_p.astype(np.float32)).astype(bf16)
    x = np.asarray(x, np.float32)
    h0 = np.asarray(h0, np.float32)
    c0 = np.asarray(c0, np.float32)
    in_maps = []
    for k in range(NCORES):
        bs = slice(B * k, B * (k + 1))
        in_maps.append(
            {
                "x": np.ascontiguousarray(x[bs, :T, :]),
                "h0": np.ascontiguousarray(h0[0, bs, :]).astype(bf16),
                "c0": np.ascontiguousarray(c0[0, bs, :]),
                "wih_p": wih_p,
                "whh_p": whh_p,
                "bias1_p": bias1_p,
                "bias2_p": bias2_p,
                "ident": ident,
            }
        )
    return in_maps


_NC_CACHE = {}


def run_hw(x, h0, c0, w_ih, w_hh, b_ih, b_hh, T=T_FULL, TC=32, trace=False):
    _ensure_paths()
    from concourse.bass_utils import run_bass_kernel_spmd

    key = (T, TC)
    if key not in _NC_CACHE:
        _NC_CACHE[key] = build_nc(T=T, TC=TC)
    nc = _NC_CACHE[key]
    in_maps = _shard_inputs(x, h0, c0, w_ih, w_hh, b_ih, b_hh, T=T)
    res = run_bass_kernel_spmd(nc, in_maps, list(range(NCORES)), trace=trace)
    hn = np.stack([res.results[k]["hn"] for k in range(NCORES)], axis=0)
    return hn.reshape(1, B_TOT, H), res


def kernel(x, h0, c0, w_ih, w_hh, b_ih, b_hh):
    out, _ = run_hw(x, h0, c0, w_ih, w_hh, b_ih, b_hh)
    return out.astype(np.float32)


def _np_reference(x, h0, c0, w_ih, w_hh, b_ih, b_hh, T=None):
    """Numpy oracle for development (matches reference.py)."""
    x = np.asarray(x, np.float64)
    if T is not None:
        x = x[:, :T, :]
    h = np.asarray(h0, np.float64)[0]
    c = np.asarray(c0, np.float64)[0]
    gx = np.einsum("bti,gi->tbg", x, np.asarray(w_ih, np.float64)) + (
        np.asarray(b_ih, np.float64) + np.asarray(b_hh, np.float64)
    )
    W = np.asarray(w_hh, np.float64)

    def sg(v):
        return 1.0 / (1.0 + np.exp(-v))

    for t in range(x.shape[1]):
        g = gx[t] + h @ W.T
        i = sg(g[:, 0:256])
        f = sg(g[:, 256:512])
        gg = np.tanh(g[:, 512:768])
        o = sg(g[:, 768:1024])
        c = f * c + i * gg
        h = o * np.tanh(c)
    return h[None].astype(np.float32)

